# revision 21
# baseline (speedup 1.0000x reference)
"""Trainium2 Bass kernel for nn_BlockModel_82678120448388.

Model: per (batch, head): 8x8 transition matrices from an MLP (normalized),
values from a second MLP, then a linear recurrence s_t = A_t s_{t-1} + v_t
over seq=2048.

Sharding: 8 cores = 4 batches x 2 sequence-halves (1024 tokens each, all 64
heads). Weights replicated; full inputs in, full output out.

Matmuls run in fp8-e4m3 DoubleRow mode (2 k-slices per instruction at 0.5
cycles/row = 4x bf16 throughput). Precision is recovered with residual
terms: L1/V1 use host-prepared residual splits of both operands
(x = x8+xr8, W = W8+Wr8; the xr8@Wr8 cross term is dropped), so their
output is bf16-quality. L2's weight side uses a host-prepared residual
(W2 = W8+Wr8) while its h side pays one e4m3 rounding (h8 is written
directly by the ReLU activation, costing nothing).

Scan: the normalized A_t are strongly contractive (product over a 16-token
window has norm ~1e-5), so the recurrence is chunk-local far below the
error tolerance. Each core runs K=16 chunk scans of C=64 tokens in
partition-parallel ((chunk, head-group) on partitions), each warmed up
with the last W=16 tokens of the previous chunk from a zero state. The
half-boundary warm-up (tokens 1008..1023) is recomputed by every core in
a cheap plain-fp8 "mini" tile; chunk 0's state after warm-up is then
blended with a per-core (mask, init) input pair so half 0 starts exactly
from a0 while half 1 keeps the recomputed state -- one SPMD program.

DMA layout: A and v for token (chunk c, pos p) are stored contiguously in
av_dram[(c,ho), p, 648] so one DMA gathers 8 scan steps; W2 is host-
re-laid-out so each (q, n) slab loads in large DMAs; x is host-striped
so each tile loads in one DMA. w18 is SBUF-resident.
"""

import numpy as np
import ml_dtypes
from contextlib import ExitStack

import concourse.bass as bass
import concourse.bacc as bacc
import concourse.tile as tile
from concourse import mybir

F32 = mybir.dt.float32
BF16 = mybir.dt.bfloat16
FP8 = mybir.dt.float8e4
AF = mybir.ActivationFunctionType
ALU = mybir.AluOpType
DR = mybir.MatmulPerfMode.DoubleRow

BS, SEQ, EMB, BD = 4, 2048, 512, 8
H = EMB // BD      # 64 global heads
HL = 64            # heads per core (all of them)
NF = HL * BD * BD  # 4096 blk feats per core
VF = HL * BD       # 512 v feats per core
HID = EMB * BD     # 4096
P = 128

N_CORES = 8
TOKC = SEQ // 2    # 1024 tokens per core

K = 16             # chunks per core
C = TOKC // K      # 64 tokens per chunk
W = 16             # warm-up tokens per chunk
NHO = P // K       # 8 head-groups on partitions
NHR = HL // NHO    # 8 heads per group in free dim
HRI = NHR * BD     # 64
AVW = NHR * BD * 9  # 648: per (c,ho,pos): (hr, i, [A row | v]) 9-wide rows
ROWW = C * AVW     # av_dram row size per (c, ho)
PPT = 8            # positions per tile per chunk (128-token tile / K chunks)

XS = 16.0          # fp8 scale on x / h
WS = 64.0          # fp8 scale on W1 / W2 / V1

QT = 384           # max token-columns per tile group
NQ = TOKC // 512   # x stripe flat layout helper (flat cols = 1024)


def _rot(tau):
    """Within-chunk position of the first token in MLP tile tau.

    Warm-up positions [C-W, C) are produced by tiles 0-1 so the scan's
    warm-up steps only depend on the first group's MLP output.
    """
    return (C - W + PPT * tau) % C


def build_nc(TOK=TOKC, scan_steps=None):
    nc = bacc.Bacc("TRN2", target_bir_lowering=False, debug=False)

    FC = TOKC  # flat x columns per core
    xs8 = nc.dram_tensor("xs8", [4 * P * FC], FP8, kind="ExternalInput")
    xsr8 = nc.dram_tensor("xsr8", [4 * P * FC], FP8, kind="ExternalInput")
    xm8 = nc.dram_tensor("xm8", [4 * P * P], FP8, kind="ExternalInput")
    xmr8 = nc.dram_tensor("xmr8", [4 * P * P], FP8, kind="ExternalInput")
    w18 = nc.dram_tensor("w18", [EMB, HID], FP8, kind="ExternalInput")
    w1r8 = nc.dram_tensor("w1r8", [EMB, HID], FP8, kind="ExternalInput")
    b1 = nc.dram_tensor("b1", [HID, 1], F32, kind="ExternalInput")   # x XS
    w28 = nc.dram_tensor("w28", [(NF // 512) * HID * 512], FP8,
                         kind="ExternalInput")
    w2r8 = nc.dram_tensor("w2r8", [(NF // 512) * HID * 512], FP8,
                          kind="ExternalInput")
    b2 = nc.dram_tensor("b2", [P, NF], BF16, kind="ExternalInput")
    v18 = nc.dram_tensor("v18", [EMB, EMB], FP8, kind="ExternalInput")
    v1r8 = nc.dram_tensor("v1r8", [EMB, EMB], FP8, kind="ExternalInput")
    c1 = nc.dram_tensor("c1", [EMB, 1], F32, kind="ExternalInput")
    v2 = nc.dram_tensor("v2", [EMB, VF], BF16, kind="ExternalInput")
    c2 = nc.dram_tensor("c2", [1, VF], BF16, kind="ExternalInput")
    smask = nc.dram_tensor("smask", [NHO, 1], BF16, kind="ExternalInput")
    s0init = nc.dram_tensor("s0init", [NHO, HRI], BF16, kind="ExternalInput")
    out = nc.dram_tensor("out", [P, C * HRI], BF16, kind="ExternalOutput")

    av_dram = nc.dram_tensor("av_scratch", [P * ROWW], BF16)
    av_mini = nc.dram_tensor("av_mini", [NHO * W * AVW], BF16)

    with ExitStack() as ctx:
        tc = ctx.enter_context(tile.TileContext(nc))
        cpool = ctx.enter_context(tc.tile_pool(name="consts", bufs=1))
        wpool = ctx.enter_context(tc.tile_pool(name="weights", bufs=1))
        xpool = ctx.enter_context(tc.tile_pool(name="xstream", bufs=2))
        hpool = ctx.enter_context(tc.tile_pool(name="hidden", bufs=1))
        hmpool = ctx.enter_context(tc.tile_pool(name="hmini", bufs=1))
        hvpool = ctx.enter_context(tc.tile_pool(name="hv", bufs=2))
        w2pool = ctx.enter_context(tc.tile_pool(name="w2stream", bufs=2))
        l1ps = ctx.enter_context(tc.tile_pool(name="l1ps", bufs=2, space="PSUM"))
        l2ps = ctx.enter_context(tc.tile_pool(name="l2ps", bufs=3, space="PSUM"))
        vps = ctx.enter_context(tc.tile_pool(name="vps", bufs=1, space="PSUM"))
        mps = ctx.enter_context(tc.tile_pool(name="mps", bufs=1, space="PSUM"))
        blkpool = ctx.enter_context(tc.tile_pool(name="blk", bufs=4))
        bmpool = ctx.enter_context(tc.tile_pool(name="blkm", bufs=1))
        pwpool = ctx.enter_context(tc.tile_pool(name="pw", bufs=2))
        atpool = ctx.enter_context(tc.tile_pool(name="at", bufs=3))
        smpool = ctx.enter_context(tc.tile_pool(name="small", bufs=4))
        agpool = ctx.enter_context(tc.tile_pool(name="agather", bufs=2))
        mopool = ctx.enter_context(tc.tile_pool(name="multout", bufs=2))
        srpool = ctx.enter_context(tc.tile_pool(name="sred", bufs=1))
        scpool = ctx.enter_context(tc.tile_pool(name="scan", bufs=1))

        # ---- constants / weights ----
        ones_s = cpool.tile([1, P], BF16, tag="ones")
        nc.vector.memset(ones_s[:], 1.0)
        b1_s = cpool.tile([P, HID // P], F32, tag="b1")
        nc.sync.dma_start(b1_s[:], b1[:].rearrange("(m p) one -> p (m one)", p=P))
        c1_s = cpool.tile([P, EMB // P], F32, tag="c1")
        nc.sync.dma_start(c1_s[:], c1[:].rearrange("(m p) one -> p (m one)", p=P))
        b2_s = cpool.tile([1, NF], BF16, tag="b2")
        c2_s = cpool.tile([1, VF], BF16, tag="c2")
        m_s = cpool.tile([NHO, 1], BF16, tag="smask")
        s0i_s = cpool.tile([NHO, HRI], BF16, tag="s0init")
        v18_s = wpool.tile([P, 4, EMB], FP8, tag="v18")
        v1r8_s = wpool.tile([P, 4, EMB], FP8, tag="v1r8")
        v2_s = wpool.tile([P, 4, VF], BF16, tag="v2")
        w18_s = wpool.tile([P, 4, HID], FP8, tag="w18s")

        # v1/w18 are read by part1(0)'s matmuls: must be emitted before them
        nc.sync.dma_start(v18_s[:], v18[:].rearrange("(k p) m -> p k m", p=P))
        nc.sync.dma_start(v1r8_s[:], v1r8[:].rearrange("(k p) m -> p k m", p=P))
        for mq in range(4):
            nc.sync.dma_start(
                w18_s[:, :, bass.ts(mq, HID // 4)],
                bass.AP(w18, mq * (HID // 4),
                        [[HID, P], [P * HID, 4], [1, HID // 4]]))

        def emit_consts():
            nc.sync.dma_start(b2_s[:], b2[:1, :])
            nc.sync.dma_start(c2_s[:], c2[:])
            nc.sync.dma_start(m_s[:], smask[:])
            nc.sync.dma_start(s0i_s[:], s0init[:])
            nc.sync.dma_start(v2_s[:], v2[:].rearrange("(k p) n -> p k n", p=P))

        # ================= scan state =================
        NS = W + C
        SW = NS * NHR * 9
        s_all = scpool.tile([P, SW], BF16, tag="sall")
        nc.vector.memset(
            bass.AP(s_all.tensor, s_all[:].offset + 8, [[SW, P], [9, NS * NHR]]),
            1.0)
        s0 = scpool.tile([P, NHR * 9], BF16, tag="s0")
        nc.vector.memset(s0[:], 0.0)
        nc.vector.memset(
            bass.AP(s0.tensor, s0[:].offset + 8, [[NHR * 9, P], [9, NHR]]), 1.0)

        agv_box = {}

        GG = 4  # scan steps per gather DMA

        def gather_group(p_first):
            """One DMA fetching GG steps' [A|v] into [P, GG*AVW]."""
            agv = agpool.tile([P, GG * AVW], BF16, tag="agv", name=f"agv{p_first}")
            if p_first < 0:
                pos = C + p_first
                # chunk-0 partitions read the recomputed boundary mini-tile
                nc.sync.dma_start(
                    agv[0:NHO, :],
                    bass.AP(av_mini, (pos - (C - W)) * AVW,
                            [[W * AVW, NHO], [1, GG * AVW]]))
                nc.sync.dma_start(
                    agv[NHO:P, :],
                    bass.AP(av_dram, pos * AVW,
                            [[NHO * ROWW, K - 1], [ROWW, NHO], [1, GG * AVW]]))
            else:
                nc.sync.dma_start(
                    agv[:], bass.AP(av_dram, p_first * AVW,
                                    [[ROWW, P], [1, GG * AVW]]))
            agv_box[p_first] = agv

        def scan_step(p):
            w = W + p
            p_first = p - (p + W) % GG
            if (p + W) % GG == 0:
                gather_group(p_first)
            agv = agv_box[p_first]
            off = ((p + W) % GG) * AVW

            if p == -W:
                sprev_t, sprev_off = s0, 0
            else:
                sprev_t, sprev_off = s_all, (w - 1) * NHR * 9
            # mo[(c,ho), (hr, i, 9)] = [A|v][i, :] * [s_prev[hr, :] | 1]
            mo = mopool.tile([P, AVW], BF16, tag="mo", name=f"mo{p}")
            nc.vector.tensor_tensor(
                bass.AP(mo.tensor, mo[:].offset, [[AVW, P], [1, AVW]]),
                bass.AP(agv.tensor, agv[:].offset + off, [[GG * AVW, P], [1, AVW]]),
                bass.AP(sprev_t.tensor, sprev_t[:].offset + sprev_off,
                        [[sprev_t.shape[1], P], [9, NHR], [0, BD], [1, 9]]),
                ALU.mult)
            with nc.allow_low_precision(reason="scan state in bf16"):
                nc.vector.tensor_reduce(
                    bass.AP(s_all.tensor, s_all[:].offset + w * NHR * 9,
                            [[SW, P], [9, NHR], [1, BD]]),
                    bass.AP(mo.tensor, mo[:].offset,
                            [[AVW, P], [72, NHR], [9, BD], [1, 9]]),
                    axis=mybir.AxisListType.X, op=ALU.add)
            if p == -1:
                # chunk 0: s <- s*mask + s0init (half 0: a0 exactly; half 1:
                # keep the recomputed boundary warm-up state)
                sl = bass.AP(s_all.tensor,
                             s_all[0:NHO, :].offset + (W - 1) * NHR * 9,
                             [[SW, NHO], [9, NHR], [1, BD]])
                nc.vector.tensor_tensor(
                    sl, sl,
                    bass.AP(m_s.tensor, m_s[:].offset,
                            [[1, NHO], [0, NHR], [0, BD]]),
                    ALU.mult)
                nc.vector.tensor_tensor(
                    sl, sl,
                    bass.AP(s0i_s.tensor, s0i_s[:].offset,
                            [[HRI, NHO], [BD, NHR], [1, BD]]),
                    ALU.add)

        def emit_out(g):
            # positions [16g, 16g+16): compact the 9-wide slots on DVE so the
            # DMA moves contiguous runs
            ot = srpool.tile([P, 16 * HRI], BF16, tag="ot", name=f"ot{g}")
            nc.vector.tensor_copy(
                bass.AP(ot.tensor, ot[:].offset,
                        [[16 * HRI, P], [HRI, 16], [BD, NHR], [1, BD]]),
                bass.AP(s_all.tensor,
                        s_all[:].offset + (W + 16 * g) * NHR * 9,
                        [[SW, P], [NHR * 9, 16], [9, NHR], [1, BD]]))
            nc.sync.dma_start(
                bass.AP(out, g * 16 * HRI, [[C * HRI, P], [1, 16 * HRI]]),
                ot[:])

        GROUPS = [(0, 3), (3, 6), (6, 8)]

        def emit_part1(g):
            """x load + L1 + v-hidden (PE + Act relu), fp8 DoubleRow."""
            t0, t1 = GROUPS[g]
            nt = t1 - t0
            QTg = nt * P
            x8q = xpool.tile([P, 4, QT], FP8, tag="x8q")
            xr8q = xpool.tile([P, 4, QT], FP8, tag="xr8q")
            nc.sync.dma_start(
                x8q[:, :, 0:QTg],
                bass.AP(xs8, t0 * P, [[FC, P], [P * FC, 4], [1, QTg]]))
            nc.sync.dma_start(
                xr8q[:, :, 0:QTg],
                bass.AP(xsr8, t0 * P, [[FC, P], [P * FC, 4], [1, QTg]]))
            h8_t = hpool.tile([P, HID // P, QT], FP8, tag="hid")
            for m in range(HID // P):
                if m % 8 == 0:
                    w1r8q = w2pool.tile([P, 4, HID // 4], FP8, tag="w1r8q",
                                        name=f"w1r8q{g}_{m // 8}")
                    nc.sync.dma_start(
                        w1r8q[:], bass.AP(w1r8, (m // 8) * (HID // 4),
                                          [[HID, P], [P * HID, 4], [1, HID // 4]]))
                ps = l1ps.tile([P, QT], F32, tag="l1")
                ms = bass.ts(m % 8, P)
                msf = bass.ts(m, P)
                for j in range(2):
                    nc.tensor.matmul(ps[:, 0:QTg], w18_s[:, 2 * j:2 * j + 2, msf],
                                     x8q[:, 2 * j:2 * j + 2, 0:QTg],
                                     start=(j == 0), stop=False, perf_mode=DR)
                for j in range(2):
                    nc.tensor.matmul(ps[:, 0:QTg], w18_s[:, 2 * j:2 * j + 2, msf],
                                     xr8q[:, 2 * j:2 * j + 2, 0:QTg],
                                     start=False, stop=False, perf_mode=DR)
                for j in range(2):
                    nc.tensor.matmul(ps[:, 0:QTg], w1r8q[:, 2 * j:2 * j + 2, ms],
                                     x8q[:, 2 * j:2 * j + 2, 0:QTg],
                                     start=False, stop=(j == 1), perf_mode=DR)
                # h8 = XS*relu(xW1+b1) in fp8: psum is at XS*WS, bias is XS*b1
                nc.scalar.activation(h8_t[:, m, 0:QTg], ps[:, 0:QTg], AF.Relu,
                                     bias=b1_s[:, m:m + 1], scale=1.0 / WS)
            hv_t = hvpool.tile([P, 4, QT], BF16, tag="hv", name=f"hv{g}")
            for m in range(4):
                ps = l1ps.tile([P, QT], F32, tag="l1")
                ms = bass.ts(m, P)
                for j in range(2):
                    nc.tensor.matmul(ps[:, 0:QTg], v18_s[:, 2 * j:2 * j + 2, ms],
                                     x8q[:, 2 * j:2 * j + 2, 0:QTg],
                                     start=(j == 0), stop=False, perf_mode=DR)
                for j in range(2):
                    nc.tensor.matmul(ps[:, 0:QTg], v18_s[:, 2 * j:2 * j + 2, ms],
                                     xr8q[:, 2 * j:2 * j + 2, 0:QTg],
                                     start=False, stop=False, perf_mode=DR)
                for j in range(2):
                    nc.tensor.matmul(ps[:, 0:QTg], v1r8_s[:, 2 * j:2 * j + 2, ms],
                                     x8q[:, 2 * j:2 * j + 2, 0:QTg],
                                     start=False, stop=(j == 1), perf_mode=DR)
                nc.scalar.activation(hv_t[:, m, 0:QTg], ps[:, 0:QTg], AF.Relu,
                                     bias=c1_s[:, m:m + 1], scale=1.0 / (XS * WS))
            return {"g": g, "t0": t0, "nt": nt, "hid": h8_t, "hv": hv_t}

        # ---------- boundary mini-tile (plain fp8; errors only touch the
        # half-boundary warm-up, which half 0 erases and half 1 only feels
        # through the contractive warm-up window) ----------
        def emit_mini_part1():
            xm8q = xpool.tile([P, 4, P], FP8, tag="xm8q")
            xmr8q = xpool.tile([P, 4, P], FP8, tag="xmr8q")
            nc.sync.dma_start(
                xm8q[:], bass.AP(xm8, 0, [[P, P], [P * P, 4], [1, P]]))
            nc.sync.dma_start(
                xmr8q[:], bass.AP(xmr8, 0, [[P, P], [P * P, 4], [1, P]]))
            h8m = hmpool.tile([P, HID // P, P], FP8, tag="hidm")
            for m in range(HID // P):
                ps = l1ps.tile([P, QT], F32, tag="l1")
                msf = bass.ts(m, P)
                for j in range(2):
                    nc.tensor.matmul(ps[:, 0:P], w18_s[:, 2 * j:2 * j + 2, msf],
                                     xm8q[:, 2 * j:2 * j + 2, :],
                                     start=(j == 0), stop=False, perf_mode=DR)
                for j in range(2):
                    nc.tensor.matmul(ps[:, 0:P], w18_s[:, 2 * j:2 * j + 2, msf],
                                     xmr8q[:, 2 * j:2 * j + 2, :],
                                     start=False, stop=(j == 1), perf_mode=DR)
                nc.scalar.activation(h8m[:, m, :], ps[:, 0:P], AF.Relu,
                                     bias=b1_s[:, m:m + 1], scale=1.0 / WS)
            hvm = hvpool.tile([P, 4, P], BF16, tag="hvm")
            for m in range(4):
                ps = l1ps.tile([P, QT], F32, tag="l1")
                ms = bass.ts(m, P)
                for j in range(2):
                    nc.tensor.matmul(ps[:, 0:P], v18_s[:, 2 * j:2 * j + 2, ms],
                                     xm8q[:, 2 * j:2 * j + 2, :],
                                     start=(j == 0), stop=(j == 1), perf_mode=DR)
                nc.scalar.activation(hvm[:, m, :], ps[:, 0:P], AF.Relu,
                                     bias=c1_s[:, m:m + 1], scale=1.0 / (XS * WS))
            return h8m, hvm

        mini = {}

        def emit_mini_l2_chunk(n, qtr, w2q8):
            h8m = mini["h8m"]
            if qtr == 0:
                mini["ps"] = mps.tile([P, 512], F32, tag="mblk",
                                      name=f"mps{n}")
                nc.tensor.matmul(mini["ps"][:], ones_s[:1, :],
                                 b2_s[:1, bass.ts(n, 512)],
                                 start=True, stop=False)
            psm = mini["ps"]
            for kp in range(4):
                k8 = 2 * kp
                k = qtr * 8 + k8
                nc.tensor.matmul(psm[:], h8m[:, k:k + 2, :],
                                 w2q8[:, k8:k8 + 2, :],
                                 start=False,
                                 stop=(qtr == 3 and kp == 3), perf_mode=DR)
            if qtr == 3:
                nc.scalar.activation(mini["blkm"][:, bass.ts(n, 512)], psm[:],
                                     AF.Identity, scale=1.0 / (XS * WS))

        def emit_mini_norm():
            blkm, hvm = mini["blkm"], mini["hvm"]
            atm = atpool.tile([P, NHO * AVW], BF16, tag="at", name="atmini")
            psv = vps.tile([P, VF], F32, tag="v")
            nc.tensor.matmul(psv[:], ones_s[:1, :], c2_s[:1, :],
                             start=True, stop=False)
            for k in range(4):
                nc.tensor.matmul(psv[:], hvm[:, k, :], v2_s[:, k, :],
                                 start=False, stop=(k == 3))
            nc.scalar.activation(
                bass.AP(atm.tensor, atm[:].offset + 8,
                        [[NHO * AVW, P], [72, HL], [9, BD]]),
                bass.AP(psv.tensor, psv[:].offset, [[VF, P], [8, HL], [1, BD]]),
                AF.Identity)
            pwm = pwpool.tile([P, NF], BF16, tag="pw", name="pwmini")
            nc.vector.tensor_tensor(pwm[:], blkm[:], blkm[:], ALU.mult)
            nc.scalar.activation(pwm[:], pwm[:], AF.Ln)
            nc.scalar.activation(pwm[:], pwm[:], AF.Exp, scale=0.6)
            pst = smpool.tile([P, HL * BD], F32, tag="pst", name="pstmini")
            with nc.allow_low_precision(reason="norm stats"):
                nc.vector.tensor_reduce(
                    pst[:].rearrange("p (h k) -> p h k", h=HL, k=BD),
                    bass.AP(pwm.tensor, pwm[:].offset,
                            [[NF, P], [64, HL], [1, BD], [8, BD]]),
                    axis=mybir.AxisListType.X, op=ALU.add)
            dm = smpool.tile([P, HL], F32, tag="dm", name="dmmini")
            nc.vector.tensor_reduce(
                dm[:].rearrange("p (h one) -> p h one", h=HL, one=1),
                pst[:].rearrange("p (h k) -> p h k", h=HL, k=BD),
                axis=mybir.AxisListType.X, op=ALU.max)
            nc.scalar.activation(dm[:], dm[:], AF.Ln)
            rch = smpool.tile([P, HL], BF16, tag="rch", name="rchmini")
            nc.scalar.activation(rch[:], dm[:], AF.Exp, scale=-1.0 / 1.2)
            nc.vector.tensor_tensor(
                bass.AP(atm.tensor, atm[:].offset,
                        [[NHO * AVW, P], [72, HL], [9, BD], [1, BD]]),
                bass.AP(blkm.tensor, blkm[:].offset,
                        [[NF, P], [64, HL], [8, BD], [1, BD]]),
                bass.AP(rch.tensor, rch[:].offset,
                        [[HL, P], [1, HL], [0, BD], [0, BD]]),
                ALU.mult)
            # only partitions 0..15 (the 16 real boundary tokens) are stored
            for ho in range(NHO):
                nc.sync.dma_start(
                    bass.AP(av_mini, ho * W * AVW, [[AVW, W], [1, AVW]]),
                    bass.AP(atm.tensor, atm[0:W, :].offset + ho * AVW,
                            [[NHO * AVW, W], [1, AVW]]))

        def emit_l2(st):
            """L2: token-major blk; W2 (+residual) streamed in quarter slabs,
            fp8 DoubleRow pairs. Group 0 also accumulates the mini tile."""
            g, nt, h8_t = st["g"], st["nt"], st["hid"]
            blks = [blkpool.tile([P, NF], BF16, tag="blk", name=f"blk{g}_{i}")
                    for i in range(nt)]
            for n in range(NF // 512):
                pss = [l2ps.tile([P, 512], F32, tag="l2", name=f"l2ps{g}_{n}_{i}")
                       for i in range(nt)]
                for ttq in range(nt):
                    # bias row: psum init = ones^T @ b2c[n-chunk]
                    nc.tensor.matmul(pss[ttq][:], ones_s[:1, :],
                                     b2_s[:1, bass.ts(n, 512)],
                                     start=True, stop=False)
                for qtr in range(4):
                    w2q8 = w2pool.tile([P, 8, 512], FP8, tag="w2n8",
                                       name=f"w2n8{g}_{n}_{qtr}")
                    nc.sync.dma_start(
                        w2q8[:], bass.AP(w28, (n * HID + 8 * qtr * P) * 512,
                                         [[512, P], [P * 512, 8], [1, 512]]))
                    w2qr8 = w2pool.tile([P, 8, 512], FP8, tag="w2nr8",
                                        name=f"w2nr8{g}_{n}_{qtr}")
                    nc.sync.dma_start(
                        w2qr8[:], bass.AP(w2r8, (n * HID + 8 * qtr * P) * 512,
                                          [[512, P], [P * 512, 8], [1, 512]]))
                    for kp in range(4):
                        k8 = 2 * kp
                        k = qtr * 8 + k8
                        last = (qtr == 3 and kp == 3)
                        for ttq in range(nt):
                            ts_ = bass.ts(ttq, P)
                            nc.tensor.matmul(pss[ttq][:],
                                             h8_t[:, k:k + 2, ts_],
                                             w2q8[:, k8:k8 + 2, :],
                                             start=False, stop=False,
                                             perf_mode=DR)
                            nc.tensor.matmul(pss[ttq][:],
                                             h8_t[:, k:k + 2, ts_],
                                             w2qr8[:, k8:k8 + 2, :],
                                             start=False, stop=last,
                                             perf_mode=DR)
                    if g == 0:
                        emit_mini_l2_chunk(n, qtr, w2q8)
                for ttq in range(nt):
                    nc.scalar.activation(blks[ttq][:, bass.ts(n, 512)], pss[ttq][:],
                                         AF.Identity, scale=1.0 / (XS * WS))
            st["blks"] = blks

        def emit_vnorm(st):
            """v2 psums + v write into at tiles (PE + small act)."""
            g, nt, hv_t = st["g"], st["nt"], st["hv"]
            ats = [atpool.tile([P, NHO * AVW], BF16, tag="at", name=f"at{g}_{i}")
                   for i in range(nt)]
            for ttq in range(nt):
                psv = vps.tile([P, VF], F32, tag="v")
                nc.tensor.matmul(psv[:], ones_s[:1, :], c2_s[:1, :],
                                 start=True, stop=False)
                for k in range(4):
                    nc.tensor.matmul(psv[:], hv_t[:, k, bass.ts(ttq, P)],
                                     v2_s[:, k, :], start=False, stop=(k == 3))
                nc.scalar.activation(
                    bass.AP(ats[ttq].tensor, ats[ttq][:].offset + 8,
                            [[NHO * AVW, P], [72, HL], [9, BD]]),
                    bass.AP(psv.tensor, psv[:].offset, [[VF, P], [8, HL], [1, BD]]),
                    AF.Identity)
            st["ats"] = ats

        def emit_norm(st):
            """|blk|^1.2 norm + A write-out per tile, with that tile's scan
            steps interleaved right after its av writes."""
            g, t0, nt = st["g"], st["t0"], st["nt"]
            blks, ats = st["blks"], st["ats"]
            lo = PPT * t0 - W
            tail = g == len(GROUPS) - 1
            rcbs = []
            for ttq in range(nt):
                pw = pwpool.tile([P, NF], BF16, tag="pw", name=f"pw{g}_{ttq}")
                nc.vector.tensor_tensor(pw[:], blks[ttq][:], blks[ttq][:],
                                        ALU.mult)
                nc.scalar.activation(pw[:], pw[:], AF.Ln)
                nc.scalar.activation(pw[:], pw[:], AF.Exp, scale=0.6)
                pst = smpool.tile([P, HL * BD], F32, tag="pst")
                with nc.allow_low_precision(reason="norm stats"):
                    nc.vector.tensor_reduce(
                        pst[:].rearrange("p (h k) -> p h k", h=HL, k=BD),
                        bass.AP(pw.tensor, pw[:].offset,
                                [[NF, P], [64, HL], [1, BD], [8, BD]]),
                        axis=mybir.AxisListType.X, op=ALU.add)
                dm = smpool.tile([P, HL], F32, tag="dm", name=f"dm{g}_{ttq}")
                nc.vector.tensor_reduce(
                    dm[:].rearrange("p (h one) -> p h one", h=HL, one=1),
                    pst[:].rearrange("p (h k) -> p h k", h=HL, k=BD),
                    axis=mybir.AxisListType.X, op=ALU.max)
                nc.scalar.activation(dm[:], dm[:], AF.Ln)
                rch = smpool.tile([P, HL], BF16, tag="rch", name=f"rch{g}_{ttq}")
                nc.scalar.activation(rch[:], dm[:], AF.Exp, scale=-1.0 / 1.2)
                rcbs.append(rch)
            for ttq in range(nt):
                tau = t0 + ttq
                at = ats[ttq]
                nc.vector.tensor_tensor(
                    bass.AP(at.tensor, at[:].offset,
                            [[NHO * AVW, P], [72, HL], [9, BD], [1, BD]]),
                    bass.AP(blks[ttq].tensor, blks[ttq][:].offset,
                            [[NF, P], [64, HL], [8, BD], [1, BD]]),
                    bass.AP(rcbs[ttq].tensor, rcbs[ttq][:].offset,
                            [[HL, P], [1, HL], [0, BD], [0, BD]]),
                    ALU.mult)
                for ho in range(NHO):
                    eng = nc.sync if ho < 5 else nc.gpsimd
                    eng.dma_start(
                        bass.AP(av_dram, ho * ROWW + _rot(tau) * AVW,
                                [[NHO * ROWW, K], [AVW, PPT], [1, AVW]]),
                        bass.AP(at.tensor, at[:].offset + ho * AVW,
                                [[NHO * AVW, P], [1, AVW]]))
                for p in range(lo + 8 * ttq, lo + 8 * ttq + 8):
                    scan_step(p)
                    if p + 1 in (16, 32, 48):
                        emit_out(p // 16)
            if tail:
                for p in range(C - W, C):
                    scan_step(p)
                emit_out(3)

        # ======== software-pipelined emission ====
        prev = None
        for g in range(len(GROUPS)):
            st = emit_part1(g)
            if g == 0:
                emit_consts()
                h8m, hvm = emit_mini_part1()
                mini["h8m"], mini["hvm"] = h8m, hvm
                mini["blkm"] = bmpool.tile([P, NF], BF16, tag="blkm",
                                           name="blkm")
            if prev is not None:
                emit_vnorm(prev)
                emit_norm(prev)
            emit_l2(st)
            if g == 0:
                emit_mini_norm()
            prev = st
        emit_vnorm(prev)
        emit_norm(prev)

    nc.compile()
    _dedup_act_tables(nc)
    return nc


def _dedup_act_tables(nc):
    """All activation funcs used here (Relu/Identity/Ln/Exp) coexist in one
    hardware table (natural_log_exp_and_others), but the compile pass picks
    first-match tables per func, thrashing 1.3us loads on every Ln<->Exp
    alternation. Point the first load at the combined table and drop the
    rest (identical semantics on hw; the interpreter treats loads as no-ops).
    """
    try:
        from concourse.hw_specs import get_activation_tables
        tables = list(get_activation_tables(nc.m.arch).items())
        used = set()
        for b in nc.main_func.blocks:
            for inst in b.instructions:
                if isinstance(inst, mybir.InstActivation):
                    used.add(inst.func)
        target = None
        for idx, (name, funcs) in enumerate(tables):
            if used <= funcs:
                target = idx
                break
        if target is None:
            return
        first = True
        for b in nc.main_func.blocks:
            drop = []
            for i, inst in enumerate(b.instructions):
                if isinstance(inst, mybir.InstLoadActFuncSet):
                    if first:
                        inst.act_func_set_id = target
                        first = False
                    else:
                        si = inst.sync_info
                        if si is not None and (len(si.on_wait) or
                                               len(si.on_update)):
                            continue  # keep sem-carrying loads
                        drop.append(i)
            for i in reversed(drop):
                del b.instructions[i]
    except Exception:
        pass


# ---------------- host side ----------------

_NC_CACHE = {}


def _get_nc(TOK=TOKC):
    if TOK not in _NC_CACHE:
        _NC_CACHE[TOK] = build_nc(TOK=TOK)
    return _NC_CACHE[TOK]


def _stripe_tokens():
    """token index (within the half) for MLP column (tau, c, j) order."""
    cols = np.zeros(TOKC, np.int64)
    i = 0
    for tau in range(TOKC // P):
        for c in range(K):
            for j in range(PPT):
                cols[i] = c * C + _rot(tau) + j
                i += 1
    return cols


_F8 = ml_dtypes.float8_e4m3fn


def _q8(a):
    """e4m3 quantize (TRN-safe clip) + residual, both e4m3."""
    hi = np.clip(a, -240, 240).astype(_F8)
    lo = np.clip(a - hi.astype(np.float32), -240, 240).astype(_F8)
    return hi, lo


def prep_shared(W1, b1, W2, b2, V1, c1, V2, c2, a0):
    bf = ml_dtypes.bfloat16
    W2r = W2.reshape(H, BD, BD, HID)
    W2c = (W2r - W2r.mean(axis=1, keepdims=True)).reshape(H * BD * BD, HID)
    b2r = b2.reshape(H, BD, BD)
    b2c = (b2r - b2r.mean(axis=1, keepdims=True)).reshape(-1)
    w18, w1r8 = _q8(np.ascontiguousarray(W1.T).astype(np.float32) * WS)
    v18, v1r8 = _q8(np.ascontiguousarray(V1.T).astype(np.float32) * WS)
    w2h = np.ascontiguousarray(W2c.T).astype(np.float32) * WS  # [HID, NF]
    w2hi, w2lo = _q8(w2h)

    def slab(a):
        return np.ascontiguousarray(
            a.reshape(HID, NF // 512, 512).transpose(1, 0, 2)).reshape(-1)

    a0h = np.asarray(a0)[0]                            # [64, 8]
    a0p = a0h.reshape(NHO, NHR, BD).reshape(NHO, HRI)  # [ho, (hr, i)]
    shared = {
        "w18": w18, "w1r8": w1r8,
        "b1": np.asarray(b1).reshape(HID, 1).astype(np.float32) * XS,
        "v18": v18, "v1r8": v1r8,
        "c1": np.asarray(c1).reshape(EMB, 1).astype(np.float32),
        "w28": slab(w2hi),
        "w2r8": slab(w2lo),
        "b2": np.ascontiguousarray(
            np.broadcast_to((b2c * XS * WS).reshape(1, NF), (P, NF))).astype(bf),
        "v2": np.ascontiguousarray(V2.T).astype(bf),
        "c2": np.asarray(c2).reshape(1, VF).astype(bf),
    }
    halves = [
        {"smask": np.zeros((NHO, 1), bf),
         "s0init": a0p.astype(bf)},
        {"smask": np.ones((NHO, 1), bf),
         "s0init": np.zeros((NHO, HRI), bf)},
    ]
    return shared, halves


def make_in_maps(x, W1, b1, W2, b2, V1, c1, V2, c2, a0):
    shared, halves = prep_shared(W1, b1, W2, b2, V1, c1, V2, c2, a0)
    cols = _stripe_tokens()
    in_maps = []
    xq_cache = {}
    for core in range(N_CORES):
        b, half = core // 2, core % 2
        m = dict(shared)
        m.update(halves[half])
        key = (b, half)
        if key not in xq_cache:
            xT = np.asarray(x)[b].T.astype(np.float32) * XS  # [EMB, SEQ]
            xst = xT[:, half * TOKC + cols]                  # striped columns
            x8, xr8 = _q8(xst)
            xm = np.tile(xT[:, TOKC - W:TOKC], (1, P // W))  # boundary tokens
            xm8, xmr8 = _q8(xm)
            xq_cache[key] = (
                np.ascontiguousarray(x8.reshape(4, P, TOKC)).reshape(-1),
                np.ascontiguousarray(xr8.reshape(4, P, TOKC)).reshape(-1),
                np.ascontiguousarray(xm8.reshape(4, P, P)).reshape(-1),
                np.ascontiguousarray(xmr8.reshape(4, P, P)).reshape(-1))
        m["xs8"], m["xsr8"], m["xm8"], m["xmr8"] = xq_cache[key]
        in_maps.append(m)
    return in_maps


def kernel(x, W1, b1, W2, b2, V1, c1, V2, c2, a0):
    from concourse import bass_utils
    nc = _get_nc(TOKC)
    in_maps = make_in_maps(x, W1, b1, W2, b2, V1, c1, V2, c2, a0)
    res = bass_utils.run_bass_kernel_spmd(nc, in_maps, core_ids=list(range(N_CORES)))
    out = np.zeros((BS, SEQ, EMB), np.float32)
    for core in range(N_CORES):
        b, half = core // 2, core % 2
        raw = res.results[core]["out"].astype(np.float32)   # [128, C*HRI]
        o = raw.reshape(K, NHO, C, NHR, BD).transpose(0, 2, 1, 3, 4)
        out[b, half * TOKC:(half + 1) * TOKC, :] = o.reshape(TOKC, EMB)
    return out


# revision 23
# speedup vs baseline: 1.1008x; 1.1008x over previous
"""Trainium2 Bass kernel for nn_BlockModel_82678120448388.

Model: per (batch, head): 8x8 transition matrices from an MLP (normalized),
values from a second MLP, then a linear recurrence s_t = A_t s_{t-1} + v_t
over seq=2048.

Sharding: 8 cores = 4 batches x 2 sequence-halves (1024 tokens each, all 64
heads). Weights replicated; full inputs in, full output out.

Matmuls run in fp8-e4m3 DoubleRow mode (2 k-slices per instruction at 0.5
cycles/row = 4x bf16 throughput). Precision is recovered with residual
terms: L1/V1 use host-prepared residual splits of both operands
(x = x8+xr8, W = W8+Wr8; the xr8@Wr8 cross term is dropped), so their
output is bf16-quality. L2's weight side uses a host-prepared residual
(W2 = W8+Wr8) while its h side pays one e4m3 rounding (h8 is written
directly by the ReLU activation, costing nothing).

Scan: the normalized A_t are strongly contractive (product over a 16-token
window has norm ~1e-5), so the recurrence is chunk-local far below the
error tolerance. Each core runs K=16 chunk scans of C=64 tokens in
partition-parallel ((chunk, head-group) on partitions), each warmed up
with the last W=16 tokens of the previous chunk from a zero state. The
half-boundary warm-up (tokens 1008..1023) is recomputed by every core in
a cheap plain-fp8 "mini" tile; chunk 0's state after warm-up is then
blended with a per-core (mask, init) input pair so half 0 starts exactly
from a0 while half 1 keeps the recomputed state -- one SPMD program.

DMA layout: A and v for token (chunk c, pos p) are stored contiguously in
av_dram[(c,ho), p, 648] so one DMA gathers 8 scan steps; W2 is host-
re-laid-out so each (q, n) slab loads in large DMAs; x is host-striped
so each tile loads in one DMA. w18 is SBUF-resident.
"""

import numpy as np
import ml_dtypes
from contextlib import ExitStack

import concourse.bass as bass
import concourse.bacc as bacc
import concourse.tile as tile
from concourse import mybir

F32 = mybir.dt.float32
BF16 = mybir.dt.bfloat16
FP8 = mybir.dt.float8e4
AF = mybir.ActivationFunctionType
ALU = mybir.AluOpType
DR = mybir.MatmulPerfMode.DoubleRow

BS, SEQ, EMB, BD = 4, 2048, 512, 8
H = EMB // BD      # 64 global heads
HL = 64            # heads per core (all of them)
NF = HL * BD * BD  # 4096 blk feats per core
VF = HL * BD       # 512 v feats per core
HID = EMB * BD     # 4096
P = 128

N_CORES = 8
TOKC = SEQ // 2    # 1024 tokens per core

K = 16             # chunks per core
C = TOKC // K      # 64 tokens per chunk
W = 16             # warm-up tokens per chunk
NHO = P // K       # 8 head-groups on partitions
NHR = HL // NHO    # 8 heads per group in free dim
HRI = NHR * BD     # 64
AVW = NHR * BD * 9  # 648: per (c,ho,pos): (hr, i, [A row | v]) 9-wide rows
ROWW = C * AVW     # av_dram row size per (c, ho)
PPT = 8            # positions per tile per chunk (128-token tile / K chunks)

XS = 16.0          # fp8 scale on x / h
WS = 64.0          # fp8 scale on W1 / W2 / V1

QT = 512           # max token-columns per tile group
NQ = TOKC // 512   # x stripe flat layout helper (flat cols = 1024)


def _rot(tau):
    """Within-chunk position of the first token in MLP tile tau.

    Warm-up positions [C-W, C) are produced by tiles 0-1 so the scan's
    warm-up steps only depend on the first group's MLP output.
    """
    return (C - W + PPT * tau) % C


def build_nc(TOK=TOKC, scan_steps=None):
    nc = bacc.Bacc("TRN2", target_bir_lowering=False, debug=False)

    FC = TOKC  # flat x columns per core
    xs8 = nc.dram_tensor("xs8", [4 * P * FC], FP8, kind="ExternalInput")
    xsr8 = nc.dram_tensor("xsr8", [4 * P * FC], FP8, kind="ExternalInput")
    xm8 = nc.dram_tensor("xm8", [4 * P * P], FP8, kind="ExternalInput")
    xmr8 = nc.dram_tensor("xmr8", [4 * P * P], FP8, kind="ExternalInput")
    w18 = nc.dram_tensor("w18", [EMB, HID], FP8, kind="ExternalInput")
    w1r8 = nc.dram_tensor("w1r8", [EMB, HID], FP8, kind="ExternalInput")
    b1 = nc.dram_tensor("b1", [HID, 1], F32, kind="ExternalInput")   # x XS
    w28 = nc.dram_tensor("w28", [(NF // 512) * HID * 512], FP8,
                         kind="ExternalInput")
    w2r8 = nc.dram_tensor("w2r8", [(NF // 512) * HID * 512], FP8,
                          kind="ExternalInput")
    b2 = nc.dram_tensor("b2", [P, NF], BF16, kind="ExternalInput")
    v18 = nc.dram_tensor("v18", [EMB, EMB], FP8, kind="ExternalInput")
    v1r8 = nc.dram_tensor("v1r8", [EMB, EMB], FP8, kind="ExternalInput")
    c1 = nc.dram_tensor("c1", [EMB, 1], F32, kind="ExternalInput")
    v2 = nc.dram_tensor("v2", [EMB, VF], BF16, kind="ExternalInput")
    c2 = nc.dram_tensor("c2", [1, VF], BF16, kind="ExternalInput")
    smask = nc.dram_tensor("smask", [NHO, 1], BF16, kind="ExternalInput")
    s0init = nc.dram_tensor("s0init", [NHO, HRI], BF16, kind="ExternalInput")
    out = nc.dram_tensor("out", [P, C * HRI], BF16, kind="ExternalOutput")

    av_dram = nc.dram_tensor("av_scratch", [P * ROWW], BF16)
    av_mini = nc.dram_tensor("av_mini", [NHO * W * AVW], BF16)

    with ExitStack() as ctx:
        tc = ctx.enter_context(tile.TileContext(nc))
        cpool = ctx.enter_context(tc.tile_pool(name="consts", bufs=1))
        wpool = ctx.enter_context(tc.tile_pool(name="weights", bufs=1))
        xpool = ctx.enter_context(tc.tile_pool(name="xstream", bufs=2))
        hpool = ctx.enter_context(tc.tile_pool(name="hidden", bufs=1))
        hmpool = ctx.enter_context(tc.tile_pool(name="hmini", bufs=1))
        hvpool = ctx.enter_context(tc.tile_pool(name="hv", bufs=2))
        w2pool = ctx.enter_context(tc.tile_pool(name="w2stream", bufs=2))
        psp = ctx.enter_context(tc.tile_pool(name="psp", bufs=8, space="PSUM"))
        blkpool = ctx.enter_context(tc.tile_pool(name="blk", bufs=4))
        bmpool = ctx.enter_context(tc.tile_pool(name="blkm", bufs=1))
        pwpool = ctx.enter_context(tc.tile_pool(name="pw", bufs=1))
        atpool = ctx.enter_context(tc.tile_pool(name="at", bufs=3))
        smpool = ctx.enter_context(tc.tile_pool(name="small", bufs=4))
        agpool = ctx.enter_context(tc.tile_pool(name="agather", bufs=2))
        mopool = ctx.enter_context(tc.tile_pool(name="multout", bufs=2))
        srpool = ctx.enter_context(tc.tile_pool(name="sred", bufs=1))
        scpool = ctx.enter_context(tc.tile_pool(name="scan", bufs=1))

        # ---- constants / weights ----
        ones_s = cpool.tile([1, P], BF16, tag="ones")
        nc.vector.memset(ones_s[:], 1.0)
        b1_s = cpool.tile([P, HID // P], F32, tag="b1")
        nc.sync.dma_start(b1_s[:], b1[:].rearrange("(m p) one -> p (m one)", p=P))
        c1_s = cpool.tile([P, EMB // P], F32, tag="c1")
        nc.sync.dma_start(c1_s[:], c1[:].rearrange("(m p) one -> p (m one)", p=P))
        b2_s = cpool.tile([1, NF], BF16, tag="b2")
        c2_s = cpool.tile([1, VF], BF16, tag="c2")
        m_s = cpool.tile([NHO, 1], BF16, tag="smask")
        s0i_s = cpool.tile([NHO, HRI], BF16, tag="s0init")
        v18_s = wpool.tile([P, 4, EMB], FP8, tag="v18")
        v1r8_s = wpool.tile([P, 4, EMB], FP8, tag="v1r8")
        v2_s = wpool.tile([P, 4, VF], BF16, tag="v2")
        w18_s = wpool.tile([P, 4, HID], FP8, tag="w18s")

        # v1/w18 are read by part1(0)'s matmuls: must be emitted before them
        nc.sync.dma_start(v18_s[:], v18[:].rearrange("(k p) m -> p k m", p=P))
        nc.sync.dma_start(v1r8_s[:], v1r8[:].rearrange("(k p) m -> p k m", p=P))
        for mq in range(4):
            nc.sync.dma_start(
                w18_s[:, :, bass.ts(mq, HID // 4)],
                bass.AP(w18, mq * (HID // 4),
                        [[HID, P], [P * HID, 4], [1, HID // 4]]))

        def emit_consts():
            nc.sync.dma_start(b2_s[:], b2[:1, :])
            nc.sync.dma_start(c2_s[:], c2[:])
            nc.sync.dma_start(m_s[:], smask[:])
            nc.sync.dma_start(s0i_s[:], s0init[:])
            nc.sync.dma_start(v2_s[:], v2[:].rearrange("(k p) n -> p k n", p=P))

        # ================= scan state =================
        NS = W + C
        SW = NS * NHR * 9
        s_all = scpool.tile([P, SW], BF16, tag="sall")
        nc.vector.memset(
            bass.AP(s_all.tensor, s_all[:].offset + 8, [[SW, P], [9, NS * NHR]]),
            1.0)
        s0 = scpool.tile([P, NHR * 9], BF16, tag="s0")
        nc.vector.memset(s0[:], 0.0)
        nc.vector.memset(
            bass.AP(s0.tensor, s0[:].offset + 8, [[NHR * 9, P], [9, NHR]]), 1.0)

        agv_box = {}

        GG = 4  # scan steps per gather DMA

        def gather_group(p_first):
            """One DMA fetching GG steps' [A|v] into [P, GG*AVW]."""
            agv = agpool.tile([P, GG * AVW], BF16, tag="agv", name=f"agv{p_first}")
            if p_first < 0:
                pos = C + p_first
                # chunk-0 partitions read the recomputed boundary mini-tile
                nc.sync.dma_start(
                    agv[0:NHO, :],
                    bass.AP(av_mini, (pos - (C - W)) * AVW,
                            [[W * AVW, NHO], [1, GG * AVW]]))
                nc.sync.dma_start(
                    agv[NHO:P, :],
                    bass.AP(av_dram, pos * AVW,
                            [[NHO * ROWW, K - 1], [ROWW, NHO], [1, GG * AVW]]))
            else:
                nc.sync.dma_start(
                    agv[:], bass.AP(av_dram, p_first * AVW,
                                    [[ROWW, P], [1, GG * AVW]]))
            agv_box[p_first] = agv

        def scan_step(p):
            w = W + p
            p_first = p - (p + W) % GG
            if (p + W) % GG == 0:
                gather_group(p_first)
            agv = agv_box[p_first]
            off = ((p + W) % GG) * AVW

            if p == -W:
                sprev_t, sprev_off = s0, 0
            else:
                sprev_t, sprev_off = s_all, (w - 1) * NHR * 9
            # mo[(c,ho), (hr, i, 9)] = [A|v][i, :] * [s_prev[hr, :] | 1]
            mo = mopool.tile([P, AVW], BF16, tag="mo", name=f"mo{p}")
            nc.vector.tensor_tensor(
                bass.AP(mo.tensor, mo[:].offset, [[AVW, P], [1, AVW]]),
                bass.AP(agv.tensor, agv[:].offset + off, [[GG * AVW, P], [1, AVW]]),
                bass.AP(sprev_t.tensor, sprev_t[:].offset + sprev_off,
                        [[sprev_t.shape[1], P], [9, NHR], [0, BD], [1, 9]]),
                ALU.mult)
            with nc.allow_low_precision(reason="scan state in bf16"):
                nc.vector.tensor_reduce(
                    bass.AP(s_all.tensor, s_all[:].offset + w * NHR * 9,
                            [[SW, P], [9, NHR], [1, BD]]),
                    bass.AP(mo.tensor, mo[:].offset,
                            [[AVW, P], [72, NHR], [9, BD], [1, 9]]),
                    axis=mybir.AxisListType.X, op=ALU.add)
            if p == -1:
                # chunk 0: s <- s*mask + s0init (half 0: a0 exactly; half 1:
                # keep the recomputed boundary warm-up state)
                sl = bass.AP(s_all.tensor,
                             s_all[0:NHO, :].offset + (W - 1) * NHR * 9,
                             [[SW, NHO], [9, NHR], [1, BD]])
                nc.vector.tensor_tensor(
                    sl, sl,
                    bass.AP(m_s.tensor, m_s[:].offset,
                            [[1, NHO], [0, NHR], [0, BD]]),
                    ALU.mult)
                nc.vector.tensor_tensor(
                    sl, sl,
                    bass.AP(s0i_s.tensor, s0i_s[:].offset,
                            [[HRI, NHO], [BD, NHR], [1, BD]]),
                    ALU.add)

        def emit_out(g):
            # positions [16g, 16g+16): compact the 9-wide slots on DVE so the
            # DMA moves contiguous runs
            ot = srpool.tile([P, 16 * HRI], BF16, tag="ot", name=f"ot{g}")
            nc.vector.tensor_copy(
                bass.AP(ot.tensor, ot[:].offset,
                        [[16 * HRI, P], [HRI, 16], [BD, NHR], [1, BD]]),
                bass.AP(s_all.tensor,
                        s_all[:].offset + (W + 16 * g) * NHR * 9,
                        [[SW, P], [NHR * 9, 16], [9, NHR], [1, BD]]))
            nc.sync.dma_start(
                bass.AP(out, g * 16 * HRI, [[C * HRI, P], [1, 16 * HRI]]),
                ot[:])

        GROUPS = [(0, 4), (4, 8)]

        def emit_part1(g):
            """x load + L1 + v-hidden (PE + Act relu), fp8 DoubleRow."""
            t0, t1 = GROUPS[g]
            nt = t1 - t0
            QTg = nt * P
            x8q = xpool.tile([P, 4, QT], FP8, tag="x8q")
            xr8q = xpool.tile([P, 4, QT], FP8, tag="xr8q")
            nc.sync.dma_start(
                x8q[:, :, 0:QTg],
                bass.AP(xs8, t0 * P, [[FC, P], [P * FC, 4], [1, QTg]]))
            nc.sync.dma_start(
                xr8q[:, :, 0:QTg],
                bass.AP(xsr8, t0 * P, [[FC, P], [P * FC, 4], [1, QTg]]))
            h8_t = hpool.tile([P, HID // P, QT], FP8, tag="hid")
            for m in range(HID // P):
                if m % 8 == 0:
                    w1r8q = w2pool.tile([P, 4, HID // 4], FP8, tag="w1r8q",
                                        name=f"w1r8q{g}_{m // 8}")
                    nc.sync.dma_start(
                        w1r8q[:], bass.AP(w1r8, (m // 8) * (HID // 4),
                                          [[HID, P], [P * HID, 4], [1, HID // 4]]))
                ps = psp.tile([P, QT], F32, tag="ps")
                ms = bass.ts(m % 8, P)
                msf = bass.ts(m, P)
                for j in range(2):
                    nc.tensor.matmul(ps[:, 0:QTg], w18_s[:, 2 * j:2 * j + 2, msf],
                                     x8q[:, 2 * j:2 * j + 2, 0:QTg],
                                     start=(j == 0), stop=False, perf_mode=DR)
                for j in range(2):
                    nc.tensor.matmul(ps[:, 0:QTg], w18_s[:, 2 * j:2 * j + 2, msf],
                                     xr8q[:, 2 * j:2 * j + 2, 0:QTg],
                                     start=False, stop=False, perf_mode=DR)
                for j in range(2):
                    nc.tensor.matmul(ps[:, 0:QTg], w1r8q[:, 2 * j:2 * j + 2, ms],
                                     x8q[:, 2 * j:2 * j + 2, 0:QTg],
                                     start=False, stop=(j == 1), perf_mode=DR)
                # h8 = XS*relu(xW1+b1) in fp8: psum is at XS*WS, bias is XS*b1
                nc.scalar.activation(h8_t[:, m, 0:QTg], ps[:, 0:QTg], AF.Relu,
                                     bias=b1_s[:, m:m + 1], scale=1.0 / WS)
            hv_t = hvpool.tile([P, 4, QT], BF16, tag="hv", name=f"hv{g}")
            for m in range(4):
                ps = psp.tile([P, QT], F32, tag="ps")
                ms = bass.ts(m, P)
                for j in range(2):
                    nc.tensor.matmul(ps[:, 0:QTg], v18_s[:, 2 * j:2 * j + 2, ms],
                                     x8q[:, 2 * j:2 * j + 2, 0:QTg],
                                     start=(j == 0), stop=False, perf_mode=DR)
                for j in range(2):
                    nc.tensor.matmul(ps[:, 0:QTg], v18_s[:, 2 * j:2 * j + 2, ms],
                                     xr8q[:, 2 * j:2 * j + 2, 0:QTg],
                                     start=False, stop=False, perf_mode=DR)
                for j in range(2):
                    nc.tensor.matmul(ps[:, 0:QTg], v1r8_s[:, 2 * j:2 * j + 2, ms],
                                     x8q[:, 2 * j:2 * j + 2, 0:QTg],
                                     start=False, stop=(j == 1), perf_mode=DR)
                nc.scalar.activation(hv_t[:, m, 0:QTg], ps[:, 0:QTg], AF.Relu,
                                     bias=c1_s[:, m:m + 1], scale=1.0 / (XS * WS))
            return {"g": g, "t0": t0, "nt": nt, "hid": h8_t, "hv": hv_t}

        # ---------- boundary mini-tile (plain fp8; errors only touch the
        # half-boundary warm-up, which half 0 erases and half 1 only feels
        # through the contractive warm-up window) ----------
        def emit_mini_part1():
            xm8q = xpool.tile([P, 4, P], FP8, tag="xm8q")
            xmr8q = xpool.tile([P, 4, P], FP8, tag="xmr8q")
            nc.sync.dma_start(
                xm8q[:], bass.AP(xm8, 0, [[P, P], [P * P, 4], [1, P]]))
            nc.sync.dma_start(
                xmr8q[:], bass.AP(xmr8, 0, [[P, P], [P * P, 4], [1, P]]))
            h8m = hmpool.tile([P, HID // P, P], FP8, tag="hidm")
            for m in range(HID // P):
                ps = psp.tile([P, QT], F32, tag="ps")
                msf = bass.ts(m, P)
                for j in range(2):
                    nc.tensor.matmul(ps[:, 0:P], w18_s[:, 2 * j:2 * j + 2, msf],
                                     xm8q[:, 2 * j:2 * j + 2, :],
                                     start=(j == 0), stop=False, perf_mode=DR)
                for j in range(2):
                    nc.tensor.matmul(ps[:, 0:P], w18_s[:, 2 * j:2 * j + 2, msf],
                                     xmr8q[:, 2 * j:2 * j + 2, :],
                                     start=False, stop=(j == 1), perf_mode=DR)
                nc.scalar.activation(h8m[:, m, :], ps[:, 0:P], AF.Relu,
                                     bias=b1_s[:, m:m + 1], scale=1.0 / WS)
            hvm = hvpool.tile([P, 4, P], BF16, tag="hvm")
            for m in range(4):
                ps = psp.tile([P, QT], F32, tag="ps")
                ms = bass.ts(m, P)
                for j in range(2):
                    nc.tensor.matmul(ps[:, 0:P], v18_s[:, 2 * j:2 * j + 2, ms],
                                     xm8q[:, 2 * j:2 * j + 2, :],
                                     start=(j == 0), stop=(j == 1), perf_mode=DR)
                nc.scalar.activation(hvm[:, m, :], ps[:, 0:P], AF.Relu,
                                     bias=c1_s[:, m:m + 1], scale=1.0 / (XS * WS))
            return h8m, hvm

        mini = {}

        def emit_mini_l2_chunk(n, qtr, w2q8):
            h8m = mini["h8m"]
            if qtr == 0:
                mini["ps"] = psp.tile([P, 512], F32, tag="ps",
                                      name=f"mps{n}")
                nc.tensor.matmul(mini["ps"][:], ones_s[:1, :],
                                 b2_s[:1, bass.ts(n, 512)],
                                 start=True, stop=False)
            psm = mini["ps"]
            for kp in range(4):
                k8 = 2 * kp
                k = qtr * 8 + k8
                nc.tensor.matmul(psm[:], h8m[:, k:k + 2, :],
                                 w2q8[:, k8:k8 + 2, :],
                                 start=False,
                                 stop=(qtr == 3 and kp == 3), perf_mode=DR)
            if qtr == 3:
                nc.scalar.activation(mini["blkm"][:, bass.ts(n, 512)], psm[:],
                                     AF.Identity, scale=1.0 / (XS * WS))

        def emit_mini_norm():
            blkm, hvm = mini["blkm"], mini["hvm"]
            atm = atpool.tile([P, NHO * AVW], BF16, tag="at", name="atmini")
            psv = psp.tile([P, VF], F32, tag="ps", name="psv")
            nc.tensor.matmul(psv[:], ones_s[:1, :], c2_s[:1, :],
                             start=True, stop=False)
            for k in range(4):
                nc.tensor.matmul(psv[:], hvm[:, k, :], v2_s[:, k, :],
                                 start=False, stop=(k == 3))
            nc.scalar.activation(
                bass.AP(atm.tensor, atm[:].offset + 8,
                        [[NHO * AVW, P], [72, HL], [9, BD]]),
                bass.AP(psv.tensor, psv[:].offset, [[VF, P], [8, HL], [1, BD]]),
                AF.Identity)
            pwm = pwpool.tile([P, NF], BF16, tag="pw", name="pwmini")
            nc.vector.tensor_tensor(pwm[:], blkm[:], blkm[:], ALU.mult)
            nc.scalar.activation(pwm[:], pwm[:], AF.Ln)
            nc.scalar.activation(pwm[:], pwm[:], AF.Exp, scale=0.6)
            pst = smpool.tile([P, HL * BD], F32, tag="pst", name="pstmini")
            with nc.allow_low_precision(reason="norm stats"):
                nc.vector.tensor_reduce(
                    pst[:].rearrange("p (h k) -> p h k", h=HL, k=BD),
                    bass.AP(pwm.tensor, pwm[:].offset,
                            [[NF, P], [64, HL], [1, BD], [8, BD]]),
                    axis=mybir.AxisListType.X, op=ALU.add)
            dm = smpool.tile([P, HL], F32, tag="dm", name="dmmini")
            nc.vector.tensor_reduce(
                dm[:].rearrange("p (h one) -> p h one", h=HL, one=1),
                pst[:].rearrange("p (h k) -> p h k", h=HL, k=BD),
                axis=mybir.AxisListType.X, op=ALU.max)
            nc.scalar.activation(dm[:], dm[:], AF.Ln)
            rch = smpool.tile([P, HL], BF16, tag="rch", name="rchmini")
            nc.scalar.activation(rch[:], dm[:], AF.Exp, scale=-1.0 / 1.2)
            nc.vector.tensor_tensor(
                bass.AP(atm.tensor, atm[:].offset,
                        [[NHO * AVW, P], [72, HL], [9, BD], [1, BD]]),
                bass.AP(blkm.tensor, blkm[:].offset,
                        [[NF, P], [64, HL], [8, BD], [1, BD]]),
                bass.AP(rch.tensor, rch[:].offset,
                        [[HL, P], [1, HL], [0, BD], [0, BD]]),
                ALU.mult)
            # only partitions 0..15 (the 16 real boundary tokens) are stored
            for ho in range(NHO):
                nc.sync.dma_start(
                    bass.AP(av_mini, ho * W * AVW, [[AVW, W], [1, AVW]]),
                    bass.AP(atm.tensor, atm[0:W, :].offset + ho * AVW,
                            [[NHO * AVW, W], [1, AVW]]))

        def emit_l2(st):
            """L2: token-major blk; W2 (+residual) streamed in quarter slabs,
            fp8 DoubleRow pairs. Group 0 also accumulates the mini tile."""
            g, nt, h8_t = st["g"], st["nt"], st["hid"]
            blks = [blkpool.tile([P, NF], BF16, tag="blk", name=f"blk{g}_{i}")
                    for i in range(nt)]
            for n in range(NF // 512):
                pss = [psp.tile([P, 512], F32, tag="ps", name=f"l2ps{g}_{n}_{i}")
                       for i in range(nt)]
                for ttq in range(nt):
                    # bias row: psum init = ones^T @ b2c[n-chunk]
                    nc.tensor.matmul(pss[ttq][:], ones_s[:1, :],
                                     b2_s[:1, bass.ts(n, 512)],
                                     start=True, stop=False)
                for qtr in range(4):
                    w2q8 = w2pool.tile([P, 8, 512], FP8, tag="w2n8",
                                       name=f"w2n8{g}_{n}_{qtr}")
                    nc.sync.dma_start(
                        w2q8[:], bass.AP(w28, (n * HID + 8 * qtr * P) * 512,
                                         [[512, P], [P * 512, 8], [1, 512]]))
                    w2qr8 = w2pool.tile([P, 8, 512], FP8, tag="w2nr8",
                                        name=f"w2nr8{g}_{n}_{qtr}")
                    nc.sync.dma_start(
                        w2qr8[:], bass.AP(w2r8, (n * HID + 8 * qtr * P) * 512,
                                          [[512, P], [P * 512, 8], [1, 512]]))
                    for kp in range(4):
                        k8 = 2 * kp
                        k = qtr * 8 + k8
                        last = (qtr == 3 and kp == 3)
                        for ttq in range(nt):
                            ts_ = bass.ts(ttq, P)
                            nc.tensor.matmul(pss[ttq][:],
                                             h8_t[:, k:k + 2, ts_],
                                             w2q8[:, k8:k8 + 2, :],
                                             start=False, stop=False,
                                             perf_mode=DR)
                            nc.tensor.matmul(pss[ttq][:],
                                             h8_t[:, k:k + 2, ts_],
                                             w2qr8[:, k8:k8 + 2, :],
                                             start=False, stop=last,
                                             perf_mode=DR)
                    if g == 0:
                        emit_mini_l2_chunk(n, qtr, w2q8)
                for ttq in range(nt):
                    nc.scalar.activation(blks[ttq][:, bass.ts(n, 512)], pss[ttq][:],
                                         AF.Identity, scale=1.0 / (XS * WS))
            st["blks"] = blks

        def emit_vnorm(st):
            """v2 psums + v write into at tiles (PE + small act)."""
            g, nt, hv_t = st["g"], st["nt"], st["hv"]
            ats = [atpool.tile([P, NHO * AVW], BF16, tag="at", name=f"at{g}_{i}")
                   for i in range(nt)]
            for ttq in range(nt):
                psv = psp.tile([P, VF], F32, tag="ps", name="psv")
                nc.tensor.matmul(psv[:], ones_s[:1, :], c2_s[:1, :],
                                 start=True, stop=False)
                for k in range(4):
                    nc.tensor.matmul(psv[:], hv_t[:, k, bass.ts(ttq, P)],
                                     v2_s[:, k, :], start=False, stop=(k == 3))
                nc.scalar.activation(
                    bass.AP(ats[ttq].tensor, ats[ttq][:].offset + 8,
                            [[NHO * AVW, P], [72, HL], [9, BD]]),
                    bass.AP(psv.tensor, psv[:].offset, [[VF, P], [8, HL], [1, BD]]),
                    AF.Identity)
            st["ats"] = ats

        def emit_norm(st):
            """|blk|^1.2 norm + A write-out per tile, with that tile's scan
            steps interleaved right after its av writes."""
            g, t0, nt = st["g"], st["t0"], st["nt"]
            blks, ats = st["blks"], st["ats"]
            lo = PPT * t0 - W
            tail = g == len(GROUPS) - 1
            rcbs = []
            for ttq in range(nt):
                pw = pwpool.tile([P, NF], BF16, tag="pw", name=f"pw{g}_{ttq}")
                nc.vector.tensor_tensor(pw[:], blks[ttq][:], blks[ttq][:],
                                        ALU.mult)
                nc.scalar.activation(pw[:], pw[:], AF.Ln)
                nc.scalar.activation(pw[:], pw[:], AF.Exp, scale=0.6)
                pst = smpool.tile([P, HL * BD], F32, tag="pst")
                with nc.allow_low_precision(reason="norm stats"):
                    nc.vector.tensor_reduce(
                        pst[:].rearrange("p (h k) -> p h k", h=HL, k=BD),
                        bass.AP(pw.tensor, pw[:].offset,
                                [[NF, P], [64, HL], [1, BD], [8, BD]]),
                        axis=mybir.AxisListType.X, op=ALU.add)
                dm = smpool.tile([P, HL], F32, tag="dm", name=f"dm{g}_{ttq}")
                nc.vector.tensor_reduce(
                    dm[:].rearrange("p (h one) -> p h one", h=HL, one=1),
                    pst[:].rearrange("p (h k) -> p h k", h=HL, k=BD),
                    axis=mybir.AxisListType.X, op=ALU.max)
                nc.scalar.activation(dm[:], dm[:], AF.Ln)
                rch = smpool.tile([P, HL], BF16, tag="rch", name=f"rch{g}_{ttq}")
                nc.scalar.activation(rch[:], dm[:], AF.Exp, scale=-1.0 / 1.2)
                rcbs.append(rch)
            for ttq in range(nt):
                tau = t0 + ttq
                at = ats[ttq]
                nc.vector.tensor_tensor(
                    bass.AP(at.tensor, at[:].offset,
                            [[NHO * AVW, P], [72, HL], [9, BD], [1, BD]]),
                    bass.AP(blks[ttq].tensor, blks[ttq][:].offset,
                            [[NF, P], [64, HL], [8, BD], [1, BD]]),
                    bass.AP(rcbs[ttq].tensor, rcbs[ttq][:].offset,
                            [[HL, P], [1, HL], [0, BD], [0, BD]]),
                    ALU.mult)
                for ho in range(NHO):
                    eng = nc.sync if ho < 5 else nc.gpsimd
                    eng.dma_start(
                        bass.AP(av_dram, ho * ROWW + _rot(tau) * AVW,
                                [[NHO * ROWW, K], [AVW, PPT], [1, AVW]]),
                        bass.AP(at.tensor, at[:].offset + ho * AVW,
                                [[NHO * AVW, P], [1, AVW]]))
                for p in range(lo + 8 * ttq, lo + 8 * ttq + 8):
                    scan_step(p)
                    if p + 1 in (16, 32, 48):
                        emit_out(p // 16)
            if tail:
                for p in range(C - W, C):
                    scan_step(p)
                emit_out(3)

        # ======== software-pipelined emission ====
        prev = None
        for g in range(len(GROUPS)):
            st = emit_part1(g)
            if g == 0:
                emit_consts()
                h8m, hvm = emit_mini_part1()
                mini["h8m"], mini["hvm"] = h8m, hvm
                mini["blkm"] = bmpool.tile([P, NF], BF16, tag="blkm",
                                           name="blkm")
            if prev is not None:
                emit_vnorm(prev)
                emit_norm(prev)
            emit_l2(st)
            if g == 0:
                emit_mini_norm()
            prev = st
        emit_vnorm(prev)
        emit_norm(prev)

    nc.compile()
    _dedup_act_tables(nc)
    return nc


def _dedup_act_tables(nc):
    """All activation funcs used here (Relu/Identity/Ln/Exp) coexist in one
    hardware table (natural_log_exp_and_others), but the compile pass picks
    first-match tables per func, thrashing 1.3us loads on every Ln<->Exp
    alternation. Point the first load at the combined table and drop the
    rest (identical semantics on hw; the interpreter treats loads as no-ops).
    """
    try:
        from concourse.hw_specs import get_activation_tables
        tables = list(get_activation_tables(nc.m.arch).items())
        used = set()
        for b in nc.main_func.blocks:
            for inst in b.instructions:
                if isinstance(inst, mybir.InstActivation):
                    used.add(inst.func)
        target = None
        for idx, (name, funcs) in enumerate(tables):
            if used <= funcs:
                target = idx
                break
        if target is None:
            return
        first = True
        for b in nc.main_func.blocks:
            drop = []
            for i, inst in enumerate(b.instructions):
                if isinstance(inst, mybir.InstLoadActFuncSet):
                    if first:
                        inst.act_func_set_id = target
                        first = False
                    else:
                        si = inst.sync_info
                        if si is not None and (len(si.on_wait) or
                                               len(si.on_update)):
                            continue  # keep sem-carrying loads
                        drop.append(i)
            for i in reversed(drop):
                del b.instructions[i]
    except Exception:
        pass


# ---------------- host side ----------------

_NC_CACHE = {}


def _get_nc(TOK=TOKC):
    if TOK not in _NC_CACHE:
        _NC_CACHE[TOK] = build_nc(TOK=TOK)
    return _NC_CACHE[TOK]


def _stripe_tokens():
    """token index (within the half) for MLP column (tau, c, j) order."""
    cols = np.zeros(TOKC, np.int64)
    i = 0
    for tau in range(TOKC // P):
        for c in range(K):
            for j in range(PPT):
                cols[i] = c * C + _rot(tau) + j
                i += 1
    return cols


_F8 = ml_dtypes.float8_e4m3fn


def _q8(a):
    """e4m3 quantize (TRN-safe clip) + residual, both e4m3."""
    hi = np.clip(a, -240, 240).astype(_F8)
    lo = np.clip(a - hi.astype(np.float32), -240, 240).astype(_F8)
    return hi, lo


def prep_shared(W1, b1, W2, b2, V1, c1, V2, c2, a0):
    bf = ml_dtypes.bfloat16
    W2r = W2.reshape(H, BD, BD, HID)
    W2c = (W2r - W2r.mean(axis=1, keepdims=True)).reshape(H * BD * BD, HID)
    b2r = b2.reshape(H, BD, BD)
    b2c = (b2r - b2r.mean(axis=1, keepdims=True)).reshape(-1)
    w18, w1r8 = _q8(np.ascontiguousarray(W1.T).astype(np.float32) * WS)
    v18, v1r8 = _q8(np.ascontiguousarray(V1.T).astype(np.float32) * WS)
    w2h = np.ascontiguousarray(W2c.T).astype(np.float32) * WS  # [HID, NF]
    w2hi, w2lo = _q8(w2h)

    def slab(a):
        return np.ascontiguousarray(
            a.reshape(HID, NF // 512, 512).transpose(1, 0, 2)).reshape(-1)

    a0h = np.asarray(a0)[0]                            # [64, 8]
    a0p = a0h.reshape(NHO, NHR, BD).reshape(NHO, HRI)  # [ho, (hr, i)]
    shared = {
        "w18": w18, "w1r8": w1r8,
        "b1": np.asarray(b1).reshape(HID, 1).astype(np.float32) * XS,
        "v18": v18, "v1r8": v1r8,
        "c1": np.asarray(c1).reshape(EMB, 1).astype(np.float32),
        "w28": slab(w2hi),
        "w2r8": slab(w2lo),
        "b2": np.ascontiguousarray(
            np.broadcast_to((b2c * XS * WS).reshape(1, NF), (P, NF))).astype(bf),
        "v2": np.ascontiguousarray(V2.T).astype(bf),
        "c2": np.asarray(c2).reshape(1, VF).astype(bf),
    }
    halves = [
        {"smask": np.zeros((NHO, 1), bf),
         "s0init": a0p.astype(bf)},
        {"smask": np.ones((NHO, 1), bf),
         "s0init": np.zeros((NHO, HRI), bf)},
    ]
    return shared, halves


def make_in_maps(x, W1, b1, W2, b2, V1, c1, V2, c2, a0):
    shared, halves = prep_shared(W1, b1, W2, b2, V1, c1, V2, c2, a0)
    cols = _stripe_tokens()
    in_maps = []
    xq_cache = {}
    for core in range(N_CORES):
        b, half = core // 2, core % 2
        m = dict(shared)
        m.update(halves[half])
        key = (b, half)
        if key not in xq_cache:
            xT = np.asarray(x)[b].T.astype(np.float32) * XS  # [EMB, SEQ]
            xst = xT[:, half * TOKC + cols]                  # striped columns
            x8, xr8 = _q8(xst)
            xm = np.tile(xT[:, TOKC - W:TOKC], (1, P // W))  # boundary tokens
            xm8, xmr8 = _q8(xm)
            xq_cache[key] = (
                np.ascontiguousarray(x8.reshape(4, P, TOKC)).reshape(-1),
                np.ascontiguousarray(xr8.reshape(4, P, TOKC)).reshape(-1),
                np.ascontiguousarray(xm8.reshape(4, P, P)).reshape(-1),
                np.ascontiguousarray(xmr8.reshape(4, P, P)).reshape(-1))
        m["xs8"], m["xsr8"], m["xm8"], m["xmr8"] = xq_cache[key]
        in_maps.append(m)
    return in_maps


def kernel(x, W1, b1, W2, b2, V1, c1, V2, c2, a0):
    from concourse import bass_utils
    nc = _get_nc(TOKC)
    in_maps = make_in_maps(x, W1, b1, W2, b2, V1, c1, V2, c2, a0)
    res = bass_utils.run_bass_kernel_spmd(nc, in_maps, core_ids=list(range(N_CORES)))
    out = np.zeros((BS, SEQ, EMB), np.float32)
    for core in range(N_CORES):
        b, half = core // 2, core % 2
        raw = res.results[core]["out"].astype(np.float32)   # [128, C*HRI]
        o = raw.reshape(K, NHO, C, NHR, BD).transpose(0, 2, 1, 3, 4)
        out[b, half * TOKC:(half + 1) * TOKC, :] = o.reshape(TOKC, EMB)
    return out


# revision 24
# speedup vs baseline: 1.1626x; 1.0562x over previous
"""Trainium2 Bass kernel for nn_BlockModel_82678120448388.

Model: per (batch, head): 8x8 transition matrices from an MLP (normalized),
values from a second MLP, then a linear recurrence s_t = A_t s_{t-1} + v_t
over seq=2048.

Sharding: 8 cores = 4 batches x 2 sequence-halves (1024 tokens each, all 64
heads). Weights replicated; full inputs in, full output out.

Matmuls run in fp8-e4m3 DoubleRow mode (2 k-slices per instruction at 0.5
cycles/row = 4x bf16 throughput). Precision is recovered with residual
terms: L1/V1 use host-prepared residual splits of both operands
(x = x8+xr8, W = W8+Wr8; the xr8@Wr8 cross term is dropped), so their
output is bf16-quality. L2's weight side uses a host-prepared residual
(W2 = W8+Wr8) while its h side pays one e4m3 rounding (h8 is written
directly by the ReLU activation, costing nothing).

Scan: the normalized A_t are strongly contractive (product over a 16-token
window has norm ~1e-5), so the recurrence is chunk-local far below the
error tolerance. Each core runs K=16 chunk scans of C=64 tokens in
partition-parallel ((chunk, head-group) on partitions), each warmed up
with the last W=16 tokens of the previous chunk from a zero state. The
half-boundary warm-up (tokens 1008..1023) is recomputed by every core in
a cheap plain-fp8 "mini" tile; chunk 0's state after warm-up is then
blended with a per-core (mask, init) input pair so half 0 starts exactly
from a0 while half 1 keeps the recomputed state -- one SPMD program.

DMA layout: A and v for token (chunk c, pos p) are stored contiguously in
av_dram[(c,ho), p, 648] so one DMA gathers 8 scan steps; W2 is host-
re-laid-out so each (q, n) slab loads in large DMAs; x is host-striped
so each tile loads in one DMA. w18 is SBUF-resident.
"""

import numpy as np
import ml_dtypes
from contextlib import ExitStack

import concourse.bass as bass
import concourse.bacc as bacc
import concourse.tile as tile
from concourse import mybir

F32 = mybir.dt.float32
BF16 = mybir.dt.bfloat16
FP8 = mybir.dt.float8e4
AF = mybir.ActivationFunctionType
ALU = mybir.AluOpType
DR = mybir.MatmulPerfMode.DoubleRow

BS, SEQ, EMB, BD = 4, 2048, 512, 8
H = EMB // BD      # 64 global heads
HL = 64            # heads per core (all of them)
NF = HL * BD * BD  # 4096 blk feats per core
VF = HL * BD       # 512 v feats per core
HID = EMB * BD     # 4096
P = 128

N_CORES = 8
TOKC = SEQ // 2    # 1024 tokens per core

K = 16             # chunks per core
C = TOKC // K      # 64 tokens per chunk
W = 16             # warm-up tokens per chunk
NHO = P // K       # 8 head-groups on partitions
NHR = HL // NHO    # 8 heads per group in free dim
HRI = NHR * BD     # 64
AVW = NHR * BD * 9  # 648: per (c,ho,pos): (hr, i, [A row | v]) 9-wide rows
ROWW = C * AVW     # av_dram row size per (c, ho)
PPT = 8            # positions per tile per chunk (128-token tile / K chunks)

XS = 16.0          # fp8 scale on x / h
WS = 64.0          # fp8 scale on W1 / W2 / V1

QT = 512           # max token-columns per tile group
NQ = TOKC // 512   # x stripe flat layout helper (flat cols = 1024)


def _rot(tau):
    """Within-chunk position of the first token in MLP tile tau.

    Warm-up positions [C-W, C) are produced by tiles 0-1 so the scan's
    warm-up steps only depend on the first group's MLP output.
    """
    return (C - W + PPT * tau) % C


def build_nc(TOK=TOKC, scan_steps=None):
    nc = bacc.Bacc("TRN2", target_bir_lowering=False, debug=False)

    FC = TOKC  # flat x columns per core
    xs8 = nc.dram_tensor("xs8", [4 * P * FC], FP8, kind="ExternalInput")
    xsr8 = nc.dram_tensor("xsr8", [4 * P * FC], FP8, kind="ExternalInput")
    xm8 = nc.dram_tensor("xm8", [4 * P * P], FP8, kind="ExternalInput")
    xmr8 = nc.dram_tensor("xmr8", [4 * P * P], FP8, kind="ExternalInput")
    w18 = nc.dram_tensor("w18", [EMB, HID], FP8, kind="ExternalInput")
    w1r8 = nc.dram_tensor("w1r8", [EMB, HID], FP8, kind="ExternalInput")
    b1 = nc.dram_tensor("b1", [HID, 1], F32, kind="ExternalInput")   # x XS
    w28 = nc.dram_tensor("w28", [(NF // 512) * HID * 512], FP8,
                         kind="ExternalInput")
    w2r8 = nc.dram_tensor("w2r8", [(NF // 512) * HID * 512], FP8,
                          kind="ExternalInput")
    b2 = nc.dram_tensor("b2", [P, NF], BF16, kind="ExternalInput")
    v18 = nc.dram_tensor("v18", [EMB, EMB], FP8, kind="ExternalInput")
    v1r8 = nc.dram_tensor("v1r8", [EMB, EMB], FP8, kind="ExternalInput")
    c1 = nc.dram_tensor("c1", [EMB, 1], F32, kind="ExternalInput")
    v2 = nc.dram_tensor("v2", [EMB, VF], BF16, kind="ExternalInput")
    c2 = nc.dram_tensor("c2", [1, VF], BF16, kind="ExternalInput")
    smask = nc.dram_tensor("smask", [NHO, 1], BF16, kind="ExternalInput")
    s0init = nc.dram_tensor("s0init", [NHO, HRI], BF16, kind="ExternalInput")
    out = nc.dram_tensor("out", [P, C * HRI], BF16, kind="ExternalOutput")

    av_dram = nc.dram_tensor("av_scratch", [P * ROWW], BF16)
    av_mini = nc.dram_tensor("av_mini", [NHO * W * AVW], BF16)

    with ExitStack() as ctx:
        tc = ctx.enter_context(tile.TileContext(nc))
        cpool = ctx.enter_context(tc.tile_pool(name="consts", bufs=1))
        wpool = ctx.enter_context(tc.tile_pool(name="weights", bufs=1))
        xpool = ctx.enter_context(tc.tile_pool(name="xstream", bufs=2))
        hpool = ctx.enter_context(tc.tile_pool(name="hidden", bufs=1))
        hmpool = ctx.enter_context(tc.tile_pool(name="hmini", bufs=1))
        hvpool = ctx.enter_context(tc.tile_pool(name="hv", bufs=2))
        w2pool = ctx.enter_context(tc.tile_pool(name="w2stream", bufs=2))
        psp = ctx.enter_context(tc.tile_pool(name="psp", bufs=8, space="PSUM"))
        blkpool = ctx.enter_context(tc.tile_pool(name="blk", bufs=4))
        bmpool = ctx.enter_context(tc.tile_pool(name="blkm", bufs=1))
        pwpool = ctx.enter_context(tc.tile_pool(name="pw", bufs=1))
        atpool = ctx.enter_context(tc.tile_pool(name="at", bufs=3))
        smpool = ctx.enter_context(tc.tile_pool(name="small", bufs=4))
        agpool = ctx.enter_context(tc.tile_pool(name="agather", bufs=2))
        mopool = ctx.enter_context(tc.tile_pool(name="multout", bufs=2))
        srpool = ctx.enter_context(tc.tile_pool(name="sred", bufs=1))
        scpool = ctx.enter_context(tc.tile_pool(name="scan", bufs=1))

        # ---- constants / weights ----
        ones_s = cpool.tile([1, P], BF16, tag="ones")
        nc.vector.memset(ones_s[:], 1.0)
        b1_s = cpool.tile([P, HID // P], F32, tag="b1")
        nc.sync.dma_start(b1_s[:], b1[:].rearrange("(m p) one -> p (m one)", p=P))
        c1_s = cpool.tile([P, EMB // P], F32, tag="c1")
        nc.sync.dma_start(c1_s[:], c1[:].rearrange("(m p) one -> p (m one)", p=P))
        b2_s = cpool.tile([1, NF], BF16, tag="b2")
        c2_s = cpool.tile([1, VF], BF16, tag="c2")
        m_s = cpool.tile([NHO, 1], BF16, tag="smask")
        s0i_s = cpool.tile([NHO, HRI], BF16, tag="s0init")
        v18_s = wpool.tile([P, 4, EMB], FP8, tag="v18")
        v1r8_s = wpool.tile([P, 4, EMB], FP8, tag="v1r8")
        v2_s = wpool.tile([P, 4, VF], BF16, tag="v2")
        w18_s = wpool.tile([P, 4, HID], FP8, tag="w18s")

        # v1/w18 are read by part1(0)'s matmuls: must be emitted before them
        nc.sync.dma_start(v18_s[:], v18[:].rearrange("(k p) m -> p k m", p=P))
        nc.sync.dma_start(v1r8_s[:], v1r8[:].rearrange("(k p) m -> p k m", p=P))
        for mq in range(4):
            nc.sync.dma_start(
                w18_s[:, :, bass.ts(mq, HID // 4)],
                bass.AP(w18, mq * (HID // 4),
                        [[HID, P], [P * HID, 4], [1, HID // 4]]))

        def emit_consts():
            nc.sync.dma_start(b2_s[:], b2[:1, :])
            nc.sync.dma_start(c2_s[:], c2[:])
            nc.sync.dma_start(m_s[:], smask[:])
            nc.sync.dma_start(s0i_s[:], s0init[:])
            nc.sync.dma_start(v2_s[:], v2[:].rearrange("(k p) n -> p k n", p=P))

        # ================= scan state =================
        NS = W + C
        SW = NS * NHR * 9
        s_all = scpool.tile([P, SW], BF16, tag="sall")
        nc.vector.memset(
            bass.AP(s_all.tensor, s_all[:].offset + 8, [[SW, P], [9, NS * NHR]]),
            1.0)
        s0 = scpool.tile([P, NHR * 9], BF16, tag="s0")
        nc.vector.memset(s0[:], 0.0)
        nc.vector.memset(
            bass.AP(s0.tensor, s0[:].offset + 8, [[NHR * 9, P], [9, NHR]]), 1.0)

        agv_box = {}

        GG = 4  # scan steps per gather DMA

        def gather_group(p_first):
            """One DMA fetching GG steps' [A|v] into [P, GG*AVW]."""
            agv = agpool.tile([P, GG * AVW], BF16, tag="agv", name=f"agv{p_first}")
            if p_first < 0:
                pos = C + p_first
                # chunk-0 partitions read the recomputed boundary mini-tile
                nc.sync.dma_start(
                    agv[0:NHO, :],
                    bass.AP(av_mini, (pos - (C - W)) * AVW,
                            [[W * AVW, NHO], [1, GG * AVW]]))
                nc.sync.dma_start(
                    agv[NHO:P, :],
                    bass.AP(av_dram, pos * AVW,
                            [[NHO * ROWW, K - 1], [ROWW, NHO], [1, GG * AVW]]))
            else:
                nc.sync.dma_start(
                    agv[:], bass.AP(av_dram, p_first * AVW,
                                    [[ROWW, P], [1, GG * AVW]]))
            agv_box[p_first] = agv

        def scan_step(p):
            w = W + p
            p_first = p - (p + W) % GG
            if (p + W) % GG == 0:
                gather_group(p_first)
            agv = agv_box[p_first]
            off = ((p + W) % GG) * AVW

            if p == -W:
                sprev_t, sprev_off = s0, 0
            else:
                sprev_t, sprev_off = s_all, (w - 1) * NHR * 9
            # mo[(c,ho), (hr, i, 9)] = [A|v][i, :] * [s_prev[hr, :] | 1]
            mo = mopool.tile([P, AVW], BF16, tag="mo", name=f"mo{p}")
            nc.vector.tensor_tensor(
                bass.AP(mo.tensor, mo[:].offset, [[AVW, P], [1, AVW]]),
                bass.AP(agv.tensor, agv[:].offset + off, [[GG * AVW, P], [1, AVW]]),
                bass.AP(sprev_t.tensor, sprev_t[:].offset + sprev_off,
                        [[sprev_t.shape[1], P], [9, NHR], [0, BD], [1, 9]]),
                ALU.mult)
            with nc.allow_low_precision(reason="scan state in bf16"):
                nc.vector.tensor_reduce(
                    bass.AP(s_all.tensor, s_all[:].offset + w * NHR * 9,
                            [[SW, P], [9, NHR], [1, BD]]),
                    bass.AP(mo.tensor, mo[:].offset,
                            [[AVW, P], [72, NHR], [9, BD], [1, 9]]),
                    axis=mybir.AxisListType.X, op=ALU.add)
            if p == -1:
                # chunk 0: s <- s*mask + s0init (half 0: a0 exactly; half 1:
                # keep the recomputed boundary warm-up state)
                sl = bass.AP(s_all.tensor,
                             s_all[0:NHO, :].offset + (W - 1) * NHR * 9,
                             [[SW, NHO], [9, NHR], [1, BD]])
                nc.vector.tensor_tensor(
                    sl, sl,
                    bass.AP(m_s.tensor, m_s[:].offset,
                            [[1, NHO], [0, NHR], [0, BD]]),
                    ALU.mult)
                nc.vector.tensor_tensor(
                    sl, sl,
                    bass.AP(s0i_s.tensor, s0i_s[:].offset,
                            [[HRI, NHO], [BD, NHR], [1, BD]]),
                    ALU.add)

        def emit_out(g):
            # positions [16g, 16g+16): compact the 9-wide slots on DVE so the
            # DMA moves contiguous runs
            ot = srpool.tile([P, 16 * HRI], BF16, tag="ot", name=f"ot{g}")
            nc.vector.tensor_copy(
                bass.AP(ot.tensor, ot[:].offset,
                        [[16 * HRI, P], [HRI, 16], [BD, NHR], [1, BD]]),
                bass.AP(s_all.tensor,
                        s_all[:].offset + (W + 16 * g) * NHR * 9,
                        [[SW, P], [NHR * 9, 16], [9, NHR], [1, BD]]))
            nc.sync.dma_start(
                bass.AP(out, g * 16 * HRI, [[C * HRI, P], [1, 16 * HRI]]),
                ot[:])

        GROUPS = [(0, 4), (4, 8)]

        def emit_part1(g):
            """x load + L1 + v-hidden (PE + Act relu), fp8 DoubleRow."""
            t0, t1 = GROUPS[g]
            nt = t1 - t0
            QTg = nt * P
            x8q = xpool.tile([P, 4, QT], FP8, tag="x8q")
            xr8q = xpool.tile([P, 4, QT], FP8, tag="xr8q")
            nc.sync.dma_start(
                x8q[:, :, 0:QTg],
                bass.AP(xs8, t0 * P, [[FC, P], [P * FC, 4], [1, QTg]]))
            nc.sync.dma_start(
                xr8q[:, :, 0:QTg],
                bass.AP(xsr8, t0 * P, [[FC, P], [P * FC, 4], [1, QTg]]))
            h8_t = hpool.tile([P, HID // P, QT], FP8, tag="hid")
            for m in range(HID // P):
                if m % 8 == 0:
                    w1r8q = w2pool.tile([P, 4, HID // 4], FP8, tag="w1r8q",
                                        name=f"w1r8q{g}_{m // 8}")
                    nc.sync.dma_start(
                        w1r8q[:], bass.AP(w1r8, (m // 8) * (HID // 4),
                                          [[HID, P], [P * HID, 4], [1, HID // 4]]))
                ps = psp.tile([P, QT], F32, tag="ps")
                ms = bass.ts(m % 8, P)
                msf = bass.ts(m, P)
                for j in range(2):
                    nc.tensor.matmul(ps[:, 0:QTg], w18_s[:, 2 * j:2 * j + 2, msf],
                                     x8q[:, 2 * j:2 * j + 2, 0:QTg],
                                     start=(j == 0), stop=False, perf_mode=DR)
                for j in range(2):
                    nc.tensor.matmul(ps[:, 0:QTg], w18_s[:, 2 * j:2 * j + 2, msf],
                                     xr8q[:, 2 * j:2 * j + 2, 0:QTg],
                                     start=False, stop=False, perf_mode=DR)
                for j in range(2):
                    nc.tensor.matmul(ps[:, 0:QTg], w1r8q[:, 2 * j:2 * j + 2, ms],
                                     x8q[:, 2 * j:2 * j + 2, 0:QTg],
                                     start=False, stop=(j == 1), perf_mode=DR)
                # h8 = XS*relu(xW1+b1) in fp8: psum is at XS*WS, bias is XS*b1
                nc.scalar.activation(h8_t[:, m, 0:QTg], ps[:, 0:QTg], AF.Relu,
                                     bias=b1_s[:, m:m + 1], scale=1.0 / WS)
            hv_t = hvpool.tile([P, 4, QT], BF16, tag="hv", name=f"hv{g}")
            for m in range(4):
                ps = psp.tile([P, QT], F32, tag="ps")
                ms = bass.ts(m, P)
                for j in range(2):
                    nc.tensor.matmul(ps[:, 0:QTg], v18_s[:, 2 * j:2 * j + 2, ms],
                                     x8q[:, 2 * j:2 * j + 2, 0:QTg],
                                     start=(j == 0), stop=False, perf_mode=DR)
                for j in range(2):
                    nc.tensor.matmul(ps[:, 0:QTg], v18_s[:, 2 * j:2 * j + 2, ms],
                                     xr8q[:, 2 * j:2 * j + 2, 0:QTg],
                                     start=False, stop=False, perf_mode=DR)
                for j in range(2):
                    nc.tensor.matmul(ps[:, 0:QTg], v1r8_s[:, 2 * j:2 * j + 2, ms],
                                     x8q[:, 2 * j:2 * j + 2, 0:QTg],
                                     start=False, stop=(j == 1), perf_mode=DR)
                nc.scalar.activation(hv_t[:, m, 0:QTg], ps[:, 0:QTg], AF.Relu,
                                     bias=c1_s[:, m:m + 1], scale=1.0 / (XS * WS))
            return {"g": g, "t0": t0, "nt": nt, "hid": h8_t, "hv": hv_t}

        # ---------- boundary mini-tile (plain fp8; errors only touch the
        # half-boundary warm-up, which half 0 erases and half 1 only feels
        # through the contractive warm-up window) ----------
        def emit_mini_part1():
            xm8q = xpool.tile([P, 4, P], FP8, tag="xm8q")
            xmr8q = xpool.tile([P, 4, P], FP8, tag="xmr8q")
            nc.sync.dma_start(
                xm8q[:], bass.AP(xm8, 0, [[P, P], [P * P, 4], [1, P]]))
            nc.sync.dma_start(
                xmr8q[:], bass.AP(xmr8, 0, [[P, P], [P * P, 4], [1, P]]))
            h8m = hmpool.tile([P, HID // P, P], FP8, tag="hidm")
            for m in range(HID // P):
                ps = psp.tile([P, QT], F32, tag="ps")
                msf = bass.ts(m, P)
                for j in range(2):
                    nc.tensor.matmul(ps[:, 0:P], w18_s[:, 2 * j:2 * j + 2, msf],
                                     xm8q[:, 2 * j:2 * j + 2, :],
                                     start=(j == 0), stop=False, perf_mode=DR)
                for j in range(2):
                    nc.tensor.matmul(ps[:, 0:P], w18_s[:, 2 * j:2 * j + 2, msf],
                                     xmr8q[:, 2 * j:2 * j + 2, :],
                                     start=False, stop=(j == 1), perf_mode=DR)
                nc.scalar.activation(h8m[:, m, :], ps[:, 0:P], AF.Relu,
                                     bias=b1_s[:, m:m + 1], scale=1.0 / WS)
            hvm = hvpool.tile([P, 4, P], BF16, tag="hvm")
            for m in range(4):
                ps = psp.tile([P, QT], F32, tag="ps")
                ms = bass.ts(m, P)
                for j in range(2):
                    nc.tensor.matmul(ps[:, 0:P], v18_s[:, 2 * j:2 * j + 2, ms],
                                     xm8q[:, 2 * j:2 * j + 2, :],
                                     start=(j == 0), stop=(j == 1), perf_mode=DR)
                nc.scalar.activation(hvm[:, m, :], ps[:, 0:P], AF.Relu,
                                     bias=c1_s[:, m:m + 1], scale=1.0 / (XS * WS))
            return h8m, hvm

        mini = {}

        def emit_mini_l2_chunk(n, qtr, w2q8):
            h8m = mini["h8m"]
            if qtr == 0:
                mini["ps"] = psp.tile([P, 512], F32, tag="ps",
                                      name=f"mps{n}")
                nc.tensor.matmul(mini["ps"][:], ones_s[:1, :],
                                 b2_s[:1, bass.ts(n, 512)],
                                 start=True, stop=False)
            psm = mini["ps"]
            for kp in range(4):
                k8 = 2 * kp
                k = qtr * 8 + k8
                nc.tensor.matmul(psm[:], h8m[:, k:k + 2, :],
                                 w2q8[:, k8:k8 + 2, :],
                                 start=False,
                                 stop=(qtr == 3 and kp == 3), perf_mode=DR)
            if qtr == 3:
                nc.scalar.activation(mini["blkm"][:, bass.ts(n, 512)], psm[:],
                                     AF.Identity, scale=1.0 / (XS * WS))

        def emit_mini_norm():
            blkm, hvm = mini["blkm"], mini["hvm"]
            atm = atpool.tile([P, NHO * AVW], BF16, tag="at", name="atmini")
            psv = psp.tile([P, VF], F32, tag="ps", name="psv")
            nc.tensor.matmul(psv[:], ones_s[:1, :], c2_s[:1, :],
                             start=True, stop=False)
            for k in range(4):
                nc.tensor.matmul(psv[:], hvm[:, k, :], v2_s[:, k, :],
                                 start=False, stop=(k == 3))
            nc.scalar.activation(
                bass.AP(atm.tensor, atm[:].offset + 8,
                        [[NHO * AVW, P], [72, HL], [9, BD]]),
                bass.AP(psv.tensor, psv[:].offset, [[VF, P], [8, HL], [1, BD]]),
                AF.Identity)
            pwm = pwpool.tile([P, NF], BF16, tag="pw", name="pwmini")
            nc.vector.tensor_tensor(pwm[:], blkm[:], blkm[:], ALU.mult)
            nc.scalar.activation(pwm[:], pwm[:], AF.Ln)
            nc.scalar.activation(pwm[:], pwm[:], AF.Exp, scale=0.6)
            pst = smpool.tile([P, HL * BD], F32, tag="pst", name="pstmini")
            with nc.allow_low_precision(reason="norm stats"):
                nc.vector.tensor_reduce(
                    pst[:].rearrange("p (h k) -> p h k", h=HL, k=BD),
                    bass.AP(pwm.tensor, pwm[:].offset,
                            [[NF, P], [64, HL], [1, BD], [8, BD]]),
                    axis=mybir.AxisListType.X, op=ALU.add)
            dm = smpool.tile([P, HL], F32, tag="dm", name="dmmini")
            nc.vector.tensor_reduce(
                dm[:].rearrange("p (h one) -> p h one", h=HL, one=1),
                pst[:].rearrange("p (h k) -> p h k", h=HL, k=BD),
                axis=mybir.AxisListType.X, op=ALU.max)
            nc.scalar.activation(dm[:], dm[:], AF.Ln)
            rch = smpool.tile([P, HL], BF16, tag="rch", name="rchmini")
            nc.scalar.activation(rch[:], dm[:], AF.Exp, scale=-1.0 / 1.2)
            nc.vector.tensor_tensor(
                bass.AP(atm.tensor, atm[:].offset,
                        [[NHO * AVW, P], [72, HL], [9, BD], [1, BD]]),
                bass.AP(blkm.tensor, blkm[:].offset,
                        [[NF, P], [64, HL], [8, BD], [1, BD]]),
                bass.AP(rch.tensor, rch[:].offset,
                        [[HL, P], [1, HL], [0, BD], [0, BD]]),
                ALU.mult)
            # only partitions 0..15 (the 16 real boundary tokens) are stored
            for ho in range(NHO):
                nc.sync.dma_start(
                    bass.AP(av_mini, ho * W * AVW, [[AVW, W], [1, AVW]]),
                    bass.AP(atm.tensor, atm[0:W, :].offset + ho * AVW,
                            [[NHO * AVW, W], [1, AVW]]))

        def emit_l2(st, interleave=None):
            """L2: token-major blk; W2 (+residual) streamed in quarter slabs,
            fp8 DoubleRow pairs. Group 0 also accumulates the mini tile.
            `interleave(n)` lets the previous group's per-tile norm+scan be
            spliced between n-chunks so Act/DVE work overlaps this group's
            PE work without hogging the queues."""
            g, nt, h8_t = st["g"], st["nt"], st["hid"]
            blks = [blkpool.tile([P, NF], BF16, tag="blk", name=f"blk{g}_{i}")
                    for i in range(nt)]
            for n in range(NF // 512):
                pss = [psp.tile([P, 512], F32, tag="ps", name=f"l2ps{g}_{n}_{i}")
                       for i in range(nt)]
                for ttq in range(nt):
                    # bias row: psum init = ones^T @ b2c[n-chunk]
                    nc.tensor.matmul(pss[ttq][:], ones_s[:1, :],
                                     b2_s[:1, bass.ts(n, 512)],
                                     start=True, stop=False)
                for qtr in range(4):
                    w2q8 = w2pool.tile([P, 8, 512], FP8, tag="w2n8",
                                       name=f"w2n8{g}_{n}_{qtr}")
                    nc.sync.dma_start(
                        w2q8[:], bass.AP(w28, (n * HID + 8 * qtr * P) * 512,
                                         [[512, P], [P * 512, 8], [1, 512]]))
                    w2qr8 = w2pool.tile([P, 8, 512], FP8, tag="w2nr8",
                                        name=f"w2nr8{g}_{n}_{qtr}")
                    nc.sync.dma_start(
                        w2qr8[:], bass.AP(w2r8, (n * HID + 8 * qtr * P) * 512,
                                          [[512, P], [P * 512, 8], [1, 512]]))
                    for kp in range(4):
                        k8 = 2 * kp
                        k = qtr * 8 + k8
                        last = (qtr == 3 and kp == 3)
                        for ttq in range(nt):
                            ts_ = bass.ts(ttq, P)
                            nc.tensor.matmul(pss[ttq][:],
                                             h8_t[:, k:k + 2, ts_],
                                             w2q8[:, k8:k8 + 2, :],
                                             start=False, stop=False,
                                             perf_mode=DR)
                            nc.tensor.matmul(pss[ttq][:],
                                             h8_t[:, k:k + 2, ts_],
                                             w2qr8[:, k8:k8 + 2, :],
                                             start=False, stop=last,
                                             perf_mode=DR)
                    if g == 0:
                        emit_mini_l2_chunk(n, qtr, w2q8)
                for ttq in range(nt):
                    nc.scalar.activation(blks[ttq][:, bass.ts(n, 512)], pss[ttq][:],
                                         AF.Identity, scale=1.0 / (XS * WS))
                if interleave is not None:
                    interleave(n)
            st["blks"] = blks

        def emit_vnorm(st):
            """v2 psums + v write into at tiles (PE + small act)."""
            g, nt, hv_t = st["g"], st["nt"], st["hv"]
            ats = [atpool.tile([P, NHO * AVW], BF16, tag="at", name=f"at{g}_{i}")
                   for i in range(nt)]
            for ttq in range(nt):
                psv = psp.tile([P, VF], F32, tag="ps", name="psv")
                nc.tensor.matmul(psv[:], ones_s[:1, :], c2_s[:1, :],
                                 start=True, stop=False)
                for k in range(4):
                    nc.tensor.matmul(psv[:], hv_t[:, k, bass.ts(ttq, P)],
                                     v2_s[:, k, :], start=False, stop=(k == 3))
                nc.scalar.activation(
                    bass.AP(ats[ttq].tensor, ats[ttq][:].offset + 8,
                            [[NHO * AVW, P], [72, HL], [9, BD]]),
                    bass.AP(psv.tensor, psv[:].offset, [[VF, P], [8, HL], [1, BD]]),
                    AF.Identity)
            st["ats"] = ats

        def norm_tile(st, ttq):
            """One tile: |blk|^1.2 norm + A write-out + its scan steps."""
            g, t0 = st["g"], st["t0"]
            blks, ats = st["blks"], st["ats"]
            lo = PPT * t0 - W
            tau = t0 + ttq
            pw = pwpool.tile([P, NF], BF16, tag="pw", name=f"pw{g}_{ttq}")
            nc.vector.tensor_tensor(pw[:], blks[ttq][:], blks[ttq][:],
                                    ALU.mult)
            nc.scalar.activation(pw[:], pw[:], AF.Ln)
            nc.scalar.activation(pw[:], pw[:], AF.Exp, scale=0.6)
            pst = smpool.tile([P, HL * BD], F32, tag="pst")
            with nc.allow_low_precision(reason="norm stats"):
                nc.vector.tensor_reduce(
                    pst[:].rearrange("p (h k) -> p h k", h=HL, k=BD),
                    bass.AP(pw.tensor, pw[:].offset,
                            [[NF, P], [64, HL], [1, BD], [8, BD]]),
                    axis=mybir.AxisListType.X, op=ALU.add)
            dm = smpool.tile([P, HL], F32, tag="dm", name=f"dm{g}_{ttq}")
            nc.vector.tensor_reduce(
                dm[:].rearrange("p (h one) -> p h one", h=HL, one=1),
                pst[:].rearrange("p (h k) -> p h k", h=HL, k=BD),
                axis=mybir.AxisListType.X, op=ALU.max)
            nc.scalar.activation(dm[:], dm[:], AF.Ln)
            rch = smpool.tile([P, HL], BF16, tag="rch", name=f"rch{g}_{ttq}")
            nc.scalar.activation(rch[:], dm[:], AF.Exp, scale=-1.0 / 1.2)
            at = ats[ttq]
            nc.vector.tensor_tensor(
                bass.AP(at.tensor, at[:].offset,
                        [[NHO * AVW, P], [72, HL], [9, BD], [1, BD]]),
                bass.AP(blks[ttq].tensor, blks[ttq][:].offset,
                        [[NF, P], [64, HL], [8, BD], [1, BD]]),
                bass.AP(rch.tensor, rch[:].offset,
                        [[HL, P], [1, HL], [0, BD], [0, BD]]),
                ALU.mult)
            for ho in range(NHO):
                eng = nc.sync if ho < 5 else nc.gpsimd
                eng.dma_start(
                    bass.AP(av_dram, ho * ROWW + _rot(tau) * AVW,
                            [[NHO * ROWW, K], [AVW, PPT], [1, AVW]]),
                    bass.AP(at.tensor, at[:].offset + ho * AVW,
                            [[NHO * AVW, P], [1, AVW]]))
            for p in range(lo + 8 * ttq, lo + 8 * ttq + 8):
                scan_step(p)
                if p + 1 in (16, 32, 48):
                    emit_out(p // 16)

        def emit_scan_tail():
            for p in range(C - W, C):
                scan_step(p)
            emit_out(3)

        # ======== software-pipelined emission ====
        st0 = emit_part1(0)
        emit_consts()
        h8m, hvm = emit_mini_part1()
        mini["h8m"], mini["hvm"] = h8m, hvm
        mini["blkm"] = bmpool.tile([P, NF], BF16, tag="blkm", name="blkm")
        emit_l2(st0)
        emit_mini_norm()
        st1 = emit_part1(1)
        emit_vnorm(st0)

        def ilv(n):
            if n % 2 == 1:
                norm_tile(st0, (n - 1) // 2)
        emit_l2(st1, interleave=ilv)
        emit_vnorm(st1)
        for t in range(4):
            norm_tile(st1, t)
        emit_scan_tail()

    nc.compile()
    _dedup_act_tables(nc)
    return nc


def _dedup_act_tables(nc):
    """All activation funcs used here (Relu/Identity/Ln/Exp) coexist in one
    hardware table (natural_log_exp_and_others), but the compile pass picks
    first-match tables per func, thrashing 1.3us loads on every Ln<->Exp
    alternation. Point the first load at the combined table and drop the
    rest (identical semantics on hw; the interpreter treats loads as no-ops).
    """
    try:
        from concourse.hw_specs import get_activation_tables
        tables = list(get_activation_tables(nc.m.arch).items())
        used = set()
        for b in nc.main_func.blocks:
            for inst in b.instructions:
                if isinstance(inst, mybir.InstActivation):
                    used.add(inst.func)
        target = None
        for idx, (name, funcs) in enumerate(tables):
            if used <= funcs:
                target = idx
                break
        if target is None:
            return
        first = True
        for b in nc.main_func.blocks:
            drop = []
            for i, inst in enumerate(b.instructions):
                if isinstance(inst, mybir.InstLoadActFuncSet):
                    if first:
                        inst.act_func_set_id = target
                        first = False
                    else:
                        si = inst.sync_info
                        if si is not None and (len(si.on_wait) or
                                               len(si.on_update)):
                            continue  # keep sem-carrying loads
                        drop.append(i)
            for i in reversed(drop):
                del b.instructions[i]
    except Exception:
        pass


# ---------------- host side ----------------

_NC_CACHE = {}


def _get_nc(TOK=TOKC):
    if TOK not in _NC_CACHE:
        _NC_CACHE[TOK] = build_nc(TOK=TOK)
    return _NC_CACHE[TOK]


def _stripe_tokens():
    """token index (within the half) for MLP column (tau, c, j) order."""
    cols = np.zeros(TOKC, np.int64)
    i = 0
    for tau in range(TOKC // P):
        for c in range(K):
            for j in range(PPT):
                cols[i] = c * C + _rot(tau) + j
                i += 1
    return cols


_F8 = ml_dtypes.float8_e4m3fn


def _q8(a):
    """e4m3 quantize (TRN-safe clip) + residual, both e4m3."""
    hi = np.clip(a, -240, 240).astype(_F8)
    lo = np.clip(a - hi.astype(np.float32), -240, 240).astype(_F8)
    return hi, lo


def prep_shared(W1, b1, W2, b2, V1, c1, V2, c2, a0):
    bf = ml_dtypes.bfloat16
    W2r = W2.reshape(H, BD, BD, HID)
    W2c = (W2r - W2r.mean(axis=1, keepdims=True)).reshape(H * BD * BD, HID)
    b2r = b2.reshape(H, BD, BD)
    b2c = (b2r - b2r.mean(axis=1, keepdims=True)).reshape(-1)
    w18, w1r8 = _q8(np.ascontiguousarray(W1.T).astype(np.float32) * WS)
    v18, v1r8 = _q8(np.ascontiguousarray(V1.T).astype(np.float32) * WS)
    w2h = np.ascontiguousarray(W2c.T).astype(np.float32) * WS  # [HID, NF]
    w2hi, w2lo = _q8(w2h)

    def slab(a):
        return np.ascontiguousarray(
            a.reshape(HID, NF // 512, 512).transpose(1, 0, 2)).reshape(-1)

    a0h = np.asarray(a0)[0]                            # [64, 8]
    a0p = a0h.reshape(NHO, NHR, BD).reshape(NHO, HRI)  # [ho, (hr, i)]
    shared = {
        "w18": w18, "w1r8": w1r8,
        "b1": np.asarray(b1).reshape(HID, 1).astype(np.float32) * XS,
        "v18": v18, "v1r8": v1r8,
        "c1": np.asarray(c1).reshape(EMB, 1).astype(np.float32),
        "w28": slab(w2hi),
        "w2r8": slab(w2lo),
        "b2": np.ascontiguousarray(
            np.broadcast_to((b2c * XS * WS).reshape(1, NF), (P, NF))).astype(bf),
        "v2": np.ascontiguousarray(V2.T).astype(bf),
        "c2": np.asarray(c2).reshape(1, VF).astype(bf),
    }
    halves = [
        {"smask": np.zeros((NHO, 1), bf),
         "s0init": a0p.astype(bf)},
        {"smask": np.ones((NHO, 1), bf),
         "s0init": np.zeros((NHO, HRI), bf)},
    ]
    return shared, halves


def make_in_maps(x, W1, b1, W2, b2, V1, c1, V2, c2, a0):
    shared, halves = prep_shared(W1, b1, W2, b2, V1, c1, V2, c2, a0)
    cols = _stripe_tokens()
    in_maps = []
    xq_cache = {}
    for core in range(N_CORES):
        b, half = core // 2, core % 2
        m = dict(shared)
        m.update(halves[half])
        key = (b, half)
        if key not in xq_cache:
            xT = np.asarray(x)[b].T.astype(np.float32) * XS  # [EMB, SEQ]
            xst = xT[:, half * TOKC + cols]                  # striped columns
            x8, xr8 = _q8(xst)
            xm = np.tile(xT[:, TOKC - W:TOKC], (1, P // W))  # boundary tokens
            xm8, xmr8 = _q8(xm)
            xq_cache[key] = (
                np.ascontiguousarray(x8.reshape(4, P, TOKC)).reshape(-1),
                np.ascontiguousarray(xr8.reshape(4, P, TOKC)).reshape(-1),
                np.ascontiguousarray(xm8.reshape(4, P, P)).reshape(-1),
                np.ascontiguousarray(xmr8.reshape(4, P, P)).reshape(-1))
        m["xs8"], m["xsr8"], m["xm8"], m["xmr8"] = xq_cache[key]
        in_maps.append(m)
    return in_maps


def kernel(x, W1, b1, W2, b2, V1, c1, V2, c2, a0):
    from concourse import bass_utils
    nc = _get_nc(TOKC)
    in_maps = make_in_maps(x, W1, b1, W2, b2, V1, c1, V2, c2, a0)
    res = bass_utils.run_bass_kernel_spmd(nc, in_maps, core_ids=list(range(N_CORES)))
    out = np.zeros((BS, SEQ, EMB), np.float32)
    for core in range(N_CORES):
        b, half = core // 2, core % 2
        raw = res.results[core]["out"].astype(np.float32)   # [128, C*HRI]
        o = raw.reshape(K, NHO, C, NHR, BD).transpose(0, 2, 1, 3, 4)
        out[b, half * TOKC:(half + 1) * TOKC, :] = o.reshape(TOKC, EMB)
    return out
